# revision 32
# baseline (speedup 1.0000x reference)
"""ChebNet (K=3, 3 layers) on trn2, node-sharded across 8 cores.

Math (per layer): out = h@(W0-W2) + P1@W1 + P2@(2*W2) + b,  P1 = L h, P2 = L P1,
L = -D^-1/2 A D^-1/2 (deg = out-degree over src).  relu after layers 0,1.

Device scheme per core (owns a contiguous slice of dst nodes):
 - features live transposed in SBUF: [128 feat (partitions), nodes (free)]
 - propagation: dma_gather rows of the (replicated, node-major) feature table
   for each edge (sorted by (block of 128 dst, bucket of <=32768 src rows)),
   128 edges -> 128 partitions; segmented sum via matmul:
     psum[f, d] += sum_e E_tile[e, f] * W2_tile[e, d]
   where W2_tile[e, d] = edge_weight if (dst_local % 128)==d else 0
   (host-precomputed, streamed from DRAM).
 - gather tables for the next prop are produced by PE-transposing each
   128-node block and AllGather-ing the node-major slices.
"""

from dataclasses import dataclass, field

import numpy as np
import ml_dtypes

import concourse.bass as bass
import concourse.bacc as bacc
import concourse.mybir as mybir
import concourse.tile as tile
from concourse import library_config
from concourse.tile import TileContext

BF16 = mybir.dt.bfloat16
F32 = mybir.dt.float32
I16 = mybir.dt.int16
AF = mybir.ActivationFunctionType
GMAXT = 4   # 128-idx tiles per dma_gather call (smaller calls pipeline better)


@dataclass
class Meta:
    N: int
    C: int          # in/hidden feature dim (must be 128)
    COUT: int
    NCORES: int
    SCB: int
    W2CHUNK: int    # blocks per w2 dma chunk
    NAG: int = 4    # allgather chunks (== src buckets)
    QRS: tuple = ()     # per-chunk per-core table rows (block-aligned)
    CHKS: tuple = ()    # per-chunk global table rows (= 8*QRS)
    QSTART: tuple = ()  # newlocal start of each quarter (block-aligned)
    OQSTART: tuple = () # orig-local start of each quarter
    CHKSTART: tuple = ()
    NPC: int = 0
    NPCP: int = 0       # padded per-core rows (NBLK*128, with holes)
    NBLK: int = 0
    NBUCK: int = 0
    perms: list = None  # [core] orig local -> newlocal (with holes)
    T: object = None              # [NBLK, NBUCK] tiles per (blk, bucket)
    sc_blocks: list = field(default_factory=list)
    TOT_TILES: int = 0
    TOT_IDXCOLS: int = 0
    ts_sc: list = None            # [sc][b] tiles in gather call
    idx_seg_col: list = None      # [sc][b] column offset of call segment
    idx_sc_col: list = None       # [sc] col base of sc idx chunk
    idx_sc_cols: list = None      # [sc] col count of sc idx chunk
    eoff: list = None             # [sc][b][blk] tile offset in (sc,b) buffer
    mm_tile: dict = None          # (blk,b,t) -> global tile index (MM order)
    GCALL: int = 8                # tiles per gather call (cell-aligned)
    calls: list = None            # [sc][b] list of (blk, t0, tk, boff)
    ccol: list = None             # [sc][b] idx col of per-call count entries

    def finalize(self):
        assert self.C == 128
        self.sc_blocks = [
            list(range(s, min(s + self.SCB, self.NBLK)))
            for s in range(0, self.NBLK, self.SCB)
        ]
        nsc = len(self.sc_blocks)
        self.ts_sc = [[0] * self.NBUCK for _ in range(nsc)]
        self.eoff = [[dict() for _ in range(self.NBUCK)] for _ in range(nsc)]
        self.mm_tile = {}
        g = 0
        for si, blks in enumerate(self.sc_blocks):
            for b in range(self.NBUCK):
                off = 0
                for blk in blks:
                    self.eoff[si][b][blk] = off
                    off += int(self.T[blk, b])
                self.ts_sc[si][b] = off
            for blk in blks:
                for b in range(self.NBUCK):
                    for t in range(int(self.T[blk, b])):
                        self.mm_tile[(blk, b, t)] = g
                        g += 1
        self.TOT_TILES = g
        # pack consecutive cells into calls of <= GCALL tiles; per-core valid
        # counts (via the count reg) trim each call to its last valid row,
        # interior-cell padding gathers row 0 harmlessly.
        # call = (cells, tk, boff); cell = (blk, t0, tcnt) tile-slice of a
        # (blk, b) group at ebuf offset boff.., tk total tiles.
        self.calls = [[[] for _ in range(self.NBUCK)] for _ in range(nsc)]
        for si, blks in enumerate(self.sc_blocks):
            for b in range(self.NBUCK):
                pieces = []
                for blk in blks:
                    Tc = int(self.T[blk, b])
                    for t0 in range(0, Tc, self.GCALL):
                        pieces.append((blk, t0, min(self.GCALL, Tc - t0)))
                group, gtiles = [], 0
                for piece in pieces:
                    if gtiles + piece[2] > self.GCALL and group:
                        self.calls[si][b].append(
                            (group, gtiles,
                             self.eoff[si][b][group[0][0]] + group[0][1]))
                        group, gtiles = [], 0
                    group.append(piece)
                    gtiles += piece[2]
                if group:
                    self.calls[si][b].append(
                        (group, gtiles,
                         self.eoff[si][b][group[0][0]] + group[0][1]))
        self.idx_seg_col = [[0] * self.NBUCK for _ in range(nsc)]
        self.ccol = [[0] * self.NBUCK for _ in range(nsc)]
        self.idx_sc_col = [0] * nsc
        self.idx_sc_cols = [0] * nsc
        col = 0
        for si in range(nsc):
            self.idx_sc_col[si] = col
            for b in range(self.NBUCK):
                self.idx_seg_col[si][b] = col
                col += self.ts_sc[si][b] * 8
                self.ccol[si][b] = col
                col += len(self.calls[si][b])
            self.idx_sc_cols[si] = col - self.idx_sc_col[si]
        self.TOT_IDXCOLS = col


def table_pos(m, v):
    """Node id -> row in the AG-chunk-major node table (post-balance)."""
    v = np.asarray(v, dtype=np.int64)
    c = v // m.NPC
    lr = v - c * m.NPC
    allperm = np.stack([m.perms[i] for i in range(m.NCORES)])
    nl = allperm[c, lr]
    q = np.searchsorted(np.asarray(m.QSTART), nl, side="right") - 1
    qs = np.asarray(m.QSTART)[q]
    return (np.asarray(m.CHKSTART)[q] + c * np.asarray(m.QRS)[q]
            + (nl - qs))


def _balance_quarter(indeg_q, nblocks, cap_reg=512, cap_ovf=1024):
    """Assign a quarter's nodes to its blocks s.t. per-(block, bucket) edge
    counts stay <= cap_reg (last block: cap_ovf). Returns block per node."""
    n_q = indeg_q.shape[0]
    order = np.argsort(-indeg_q.sum(1), kind="stable")
    cnt = np.zeros((nblocks, 4), np.int64)
    nn = np.zeros(nblocks, np.int64)
    caps = np.full((nblocks, 4), cap_reg, np.int64)
    caps[-1] = cap_ovf
    assign = np.empty(n_q, np.int64)
    for i in order:
        d = indeg_q[i]
        tot = cnt + d
        feas = (nn < 128) & (tot <= caps).all(1)
        score = (tot / caps).max(1)
        if feas.any():
            score = np.where(feas, score, np.inf)
        else:
            score = np.where(nn < 128, score, np.inf)
        blkk = int(np.argmin(score))
        assign[i] = blkk
        cnt[blkk] += d
        nn[blkk] += 1
    return assign


def make_meta(N, C, COUT, ncores, edge_index, scb=4, w2chunk=4,
              nag=4, gcall=8, balance=True, nblk_pad=6):
    m = Meta(N=N, C=C, COUT=COUT, NCORES=ncores, SCB=scb,
             W2CHUNK=w2chunk, NAG=nag, GCALL=gcall)
    assert nag == 4
    m.NPC = N // ncores
    assert m.NPC * ncores == N
    # pad the block count: hole slack lets the balancer hit <=512
    # edges per (block, bucket) cell (T=4, two cells per gather call)
    m.NBLK = (m.NPC + 127) // 128 + (nblk_pad if balance else 0)
    m.NBLK += (-m.NBLK) % nag
    m.NPCP = m.NBLK * 128
    m.NBUCK = nag
    bq = m.NBLK // nag
    nb = [bq] * nag
    m.QRS = tuple(x * 128 for x in nb)
    m.CHKS = tuple(ncores * x for x in m.QRS)
    m.QSTART = tuple(int(x) for x in np.concatenate(
        [[0], np.cumsum(m.QRS)]))[:5]
    m.CHKSTART = tuple(int(x) for x in np.concatenate(
        [[0], np.cumsum(m.CHKS)]))[:5]
    # orig-local quarter boundaries: equal shares of the real rows
    m.OQSTART = tuple(min(i * ((m.NPC + nag - 1) // nag), m.NPC)
                      for i in range(5))
    assert max(m.CHKS) <= 32768

    src = np.asarray(edge_index[0], dtype=np.int64)
    dst = np.asarray(edge_index[1], dtype=np.int64)
    oq_bounds = np.asarray(m.OQSTART[1:4])
    src_lr = src - (src // m.NPC) * m.NPC
    qsrc = np.searchsorted(oq_bounds, src_lr, side="right")  # 0..3

    # per (dst node, src quarter) in-degree
    indeg = np.bincount(dst * 4 + qsrc, minlength=N * 4).reshape(N, 4)

    m.perms = []
    for c in range(ncores):
        perm = np.empty(m.NPC, dtype=np.int64)
        for q in range(4):
            lo, hi = m.OQSTART[q], m.OQSTART[q + 1]
            nodes = np.arange(c * m.NPC + lo, c * m.NPC + hi)
            if balance:
                assign = _balance_quarter(indeg[nodes], nb[q])
            else:
                assign = (np.arange(hi - lo)) // 128
            # slot nodes within their assigned blocks
            slot = np.zeros(hi - lo, dtype=np.int64)
            for blkk in range(nb[q]):
                sel = np.nonzero(assign == blkk)[0]
                assert len(sel) <= 128, (c, q, blkk, len(sel))
                slot[sel] = np.arange(len(sel))
            perm[lo:hi] = m.QSTART[q] + assign * 128 + slot
        m.perms.append(perm)

    core = dst // m.NPC
    nl_d = np.stack(m.perms)[core, dst - core * m.NPC]
    blk = nl_d // 128
    lin = (core * m.NBLK + blk) * m.NBUCK + qsrc
    cnt = np.bincount(lin, minlength=ncores * m.NBLK * m.NBUCK).reshape(
        ncores, m.NBLK, m.NBUCK)
    m.T = np.ceil(cnt / 128.0).astype(np.int64).max(axis=0)
    m.finalize()
    return m


def prep_inputs(meta, x, edge_index, Ws, bs, table_dtype=ml_dtypes.bfloat16, w2_mode="stream"):
    """Returns per-core input dict list."""
    m = meta
    N, C = m.N, m.C
    src = np.asarray(edge_index[0], dtype=np.int64)
    dst = np.asarray(edge_index[1], dtype=np.int64)
    deg = np.bincount(src, minlength=N).astype(np.float64)
    dinv = np.where(deg > 0, 1.0 / np.sqrt(np.maximum(deg, 1e-30)), 0.0)
    w = (-(dinv[src] * dinv[dst])).astype(np.float32)

    x = np.asarray(x, dtype=np.float32)
    shared = {}
    tpos = table_pos(m, np.arange(N, dtype=np.int64))
    xt_tab = np.zeros((m.CHKSTART[4], C), dtype=table_dtype)
    # table rows are pre-scaled by dinv[src]; the -dinv[dst] side is applied
    # per-partition when prop outputs leave PSUM. The streamed "one-hot" is
    # then an exact 0/1 indicator (fp8-representable).
    xt_tab[tpos] = (x * dinv[:, None].astype(np.float32)).astype(table_dtype)
    shared["x_table"] = np.ascontiguousarray(xt_tab)
    shared["ident"] = np.eye(128, dtype=table_dtype)
    shared["iotat"] = np.ascontiguousarray(
        np.tile(np.arange(128, dtype=np.float32), (128, 1)).astype(table_dtype))
    for l in range(3):
        W = np.asarray(Ws[l], dtype=np.float32)
        shared[f"wA{l}"] = np.ascontiguousarray((W[0] - W[2]).astype(table_dtype))
        shared[f"wB{l}"] = np.ascontiguousarray(W[1].astype(table_dtype))
        shared[f"wC{l}"] = np.ascontiguousarray((2.0 * W[2]).astype(table_dtype))
        bias = np.zeros((128, 1), dtype=np.float32)
        bias[: bs[l].shape[0], 0] = np.asarray(bs[l], dtype=np.float32)
        shared[f"bias{l}"] = bias

    core = dst // m.NPC
    srcpos = table_pos(m, src)
    chk_bounds = np.asarray(m.CHKSTART[1:4])
    chk_start = np.asarray(m.CHKSTART[:4])
    per_core = []
    for c in range(m.NCORES):
        sel = np.nonzero(core == c)[0]
        s_c = srcpos[sel]
        nl_c = m.perms[c][dst[sel] - c * m.NPC]
        w_c = w[sel]
        blk_c = nl_c // 128
        buck_c = np.searchsorted(chk_bounds, s_c, side="right")
        dcol_c = nl_c % 128
        lidx_c = s_c - chk_start[buck_c]

        order = np.lexsort((buck_c, blk_c))
        blk_s, buck_s = blk_c[order], buck_c[order]
        lidx_s, w_s, dcol_s = lidx_c[order], w_c[order], dcol_c[order]
        grp = blk_s * m.NBUCK + buck_s
        starts = np.searchsorted(grp, np.arange(m.NBLK * m.NBUCK), side="left")
        ends = np.searchsorted(grp, np.arange(m.NBLK * m.NBUCK), side="right")
        ent = {}
        for blk in range(m.NBLK):
            for b in range(m.NBUCK):
                t = int(m.T[blk, b])
                if t == 0:
                    continue
                gid = blk * m.NBUCK + b
                s0, s1 = int(starts[gid]), int(ends[gid])
                L = t * 128
                n = s1 - s0
                assert n <= L, (n, L, blk, b)
                ei = np.full(L, -1, dtype=np.int16)
                ew = np.zeros(L, dtype=np.float32)
                ed = np.zeros(L, dtype=np.int64)
                ei[:n] = lidx_s[s0:s1].astype(np.int16)
                ew[:n] = w_s[s0:s1]
                ed[:n] = dcol_s[s0:s1]
                ent[(blk, b)] = (ei, ew, ed, n)

        # w2 stream [128, TOT_TILES*128] in MM order (stream mode) or
        # per-tile (off, w) scalars [128, 2*TOT_TILES] (dve mode)
        if w2_mode == "stream":
            w2 = np.zeros((128, m.TOT_TILES * 128), dtype=np.float32)
            p128 = np.arange(128)
            for (blk, b), (ei, ew, ed, n) in ent.items():
                for ti in range(int(m.T[blk, b])):
                    g = m.mm_tile[(blk, b, ti)]
                    valid = (np.arange(ti * 128, (ti + 1) * 128) < n)
                    w2[p128, g * 128 + ed[ti * 128:(ti + 1) * 128]] = \
                        valid.astype(np.float32)
        else:
            w2 = np.zeros((128, m.TOT_TILES * 2), dtype=np.float32)
            for (blk, b), (ei, ew, ed, n) in ent.items():
                for ti in range(int(m.T[blk, b])):
                    g = m.mm_tile[(blk, b, ti)]
                    w2[:, 2 * g] = ed[ti * 128:(ti + 1) * 128]
                    w2[:, 2 * g + 1] = ew[ti * 128:(ti + 1) * 128]
        # per-call valid counts: trim to the call's last valid row; padding
        # before that point gathers row 0 (idx 0), after it stays -1 so the
        # ucode's trailing-negative trim skips it per-core.
        call_cnt = {}
        for si in range(len(m.sc_blocks)):
            for b in range(m.NBUCK):
                for j, (cells, tk, boff) in enumerate(m.calls[si][b]):
                    cnt_j = 0
                    for (blk, t0, tcnt) in cells:
                        if (blk, b) not in ent:
                            continue
                        n = ent[(blk, b)][3]
                        v = min(max(n - t0 * 128, 0), tcnt * 128)
                        if v > 0:
                            off_rel = (m.eoff[si][b][blk] + t0 - boff) * 128
                            cnt_j = off_rel + v
                    if cnt_j == 0:
                        blk0, t00, _ = cells[0]
                        if (blk0, b) in ent:
                            ent[(blk0, b)][0][t00 * 128] = 0
                        cnt_j = 1
                    else:
                        for (blk, t0, tcnt) in cells:
                            if (blk, b) not in ent:
                                continue
                            ei, _, _, n = ent[(blk, b)]
                            off_rel = (m.eoff[si][b][blk] + t0 - boff) * 128
                            v = min(max(n - t0 * 128, 0), tcnt * 128)
                            lo = v
                            hi = min(tcnt * 128, cnt_j - off_rel)
                            if hi > lo:
                                ei[t0 * 128 + lo: t0 * 128 + hi] = 0
                    call_cnt[(si, b, j)] = cnt_j
        # idx stream [128, TOT_IDXCOLS] in gather-call order
        idxs = np.zeros((128, max(1, m.TOT_IDXCOLS)), dtype=np.int16)
        for si, blks in enumerate(m.sc_blocks):
            for b in range(m.NBUCK):
                seg = [ent[(blk, b)][0] for blk in blks if (blk, b) in ent]
                if seg:
                    seg = np.concatenate(seg)
                    cols = seg.reshape(-1, 16).T  # [16, L/16]
                    c0 = m.idx_seg_col[si][b]
                    idxs[:, c0: c0 + cols.shape[1]] = np.tile(cols, (8, 1))
                for j in range(len(m.calls[si][b])):
                    idxs[:, m.ccol[si][b] + j] = np.int16(
                        call_cnt[(si, b, j)])

        xT = np.zeros((128, m.NBLK * 128), dtype=table_dtype)
        xT[:C, m.perms[c]] = x[c * m.NPC:(c + 1) * m.NPC, :].T.astype(
            table_dtype)

        dv = np.zeros(m.NBLK * 128, dtype=np.float32)
        dv[m.perms[c]] = dinv[c * m.NPC:(c + 1) * m.NPC].astype(np.float32)
        dv = dv.reshape(m.NBLK, 128).T  # [dcol, blk]
        dinv_blk = np.ascontiguousarray(dv)
        dinvn_blk = np.ascontiguousarray(-dv)
        dinvsq_blk = np.ascontiguousarray(-dv * dv)

        d = dict(shared)
        fp8 = mybir.dt.np(mybir.dt.float8e4)
        d["w2"] = np.ascontiguousarray(
            w2.astype(fp8 if w2_mode == "stream" else np.float32))
        d["idxs"] = idxs
        d["xT"] = xT
        d["dinv_blk"] = dinv_blk
        d["dinvn_blk"] = dinvn_blk
        d["dinvsq_blk"] = dinvsq_blk
        per_core.append(d)
    return per_core


def build_nc(meta, table_mybir_dt=BF16, repeat=1, skip=(), ncores_override=None, gmaxt=None, nqueues=4, ebuf_bufs=5, ebuf_bufs_last=3, idx_bufs=5, w2_mode="stream", scratch=16384):
    m = meta
    ncores = ncores_override or m.NCORES
    gmaxt = gmaxt or GMAXT
    TD = table_mybir_dt
    nc = bacc.Bacc("TRN2", target_bir_lowering=False, debug=False,
                   num_devices=ncores, num_swdge_queues=4,
                   dynamic_dma_scratch_size=scratch)

    x_table = nc.dram_tensor("x_table", [m.CHKSTART[4], m.C], TD,
                             kind="ExternalInput")
    xT_in = nc.dram_tensor("xT", [128, m.NBLK * 128], TD, kind="ExternalInput")
    idx_in = nc.dram_tensor("idxs", [128, max(1, m.TOT_IDXCOLS)], I16,
                            kind="ExternalInput")
    assert w2_mode == "stream"
    FP8 = mybir.dt.float8e4
    w2_in = nc.dram_tensor("w2", [128, m.TOT_TILES * 128], FP8,
                           kind="ExternalInput")
    dinv_in = nc.dram_tensor("dinv_blk", [128, m.NBLK], F32,
                             kind="ExternalInput")
    dinvn_in = nc.dram_tensor("dinvn_blk", [128, m.NBLK], F32,
                              kind="ExternalInput")
    dinvsq_in = nc.dram_tensor("dinvsq_blk", [128, m.NBLK], F32,
                               kind="ExternalInput")
    iota_in = nc.dram_tensor("iotat", [128, 128], TD, kind="ExternalInput")
    ident_in = nc.dram_tensor("ident", [128, 128], TD, kind="ExternalInput")
    wd_in, bias_in = {}, {}
    for l in range(3):
        co = m.COUT if l == 2 else m.C
        for nm in ("A", "B", "C"):
            wd_in[(l, nm)] = nc.dram_tensor(f"w{nm}{l}", [128, co], TD,
                                            kind="ExternalInput")
        bias_in[l] = nc.dram_tensor(f"bias{l}", [128, 1], F32,
                                    kind="ExternalInput")
    out_dram = nc.dram_tensor("outT", [m.COUT, m.NBLK * 128], F32,
                              kind="ExternalOutput")

    groups = [list(range(ncores))]

    with TileContext(nc) as tc:
        with (
            tc.tile_pool(name="const", bufs=1) as constp,
            tc.tile_pool(name="feat", bufs=1) as featp,
            tc.tile_pool(name="idxp", bufs=idx_bufs) as idxp,
            tc.tile_pool(name="w2p", bufs=(3 if w2_mode != "stream" else 2)) as w2p,
            tc.tile_pool(name="e0", bufs=ebuf_bufs) as ep0,
            tc.tile_pool(name="e1", bufs=ebuf_bufs) as ep1,
            tc.tile_pool(name="e2", bufs=ebuf_bufs) as ep2,
            tc.tile_pool(name="e3", bufs=ebuf_bufs_last) as ep3,
            tc.tile_pool(name="stage", bufs=4) as stagep,
            tc.tile_pool(name="w2t", bufs=(6 if w2_mode != "stream" else 1)) as w2tp,
            tc.tile_pool(name="acc", bufs=4, space="PSUM") as accp,
            tc.tile_pool(name="tp", bufs=2, space="PSUM") as tpp,
            tc.tile_pool(name="dn", bufs=2, space="PSUM") as dnp,
            tc.tile_pool(name="dram", bufs=1, space="DRAM") as dramp,
        ):
            epools = [ep0, ep1, ep2, ep3]

            ident = constp.tile([128, 128], TD)
            nc.sync.dma_start(ident[:], ident_in[:, :])
            iota_sb = constp.tile([128, 128], TD)
            nc.sync.dma_start(iota_sb[:], iota_in[:, :])
            dinv_sb = constp.tile([128, m.NBLK], F32, tag="dinv")
            nc.sync.dma_start(dinv_sb[:], dinv_in[:, :])
            dinvn_sb = constp.tile([128, m.NBLK], F32, tag="dinvn")
            nc.sync.dma_start(dinvn_sb[:], dinvn_in[:, :])
            dinvsq_sb = constp.tile([128, m.NBLK], F32, tag="dinvsq")
            nc.sync.dma_start(dinvsq_sb[:], dinvsq_in[:, :])
            wd_sb, bias_sb = {}, {}
            for l in range(3):
                co = m.COUT if l == 2 else m.C
                for nm in ("A", "B", "C"):
                    t = constp.tile([128, co], TD, tag=f"w{nm}{l}")
                    nc.sync.dma_start(t[:], wd_in[(l, nm)][:, :])
                    wd_sb[(l, nm)] = t
                bt = constp.tile([128, 1], F32, tag=f"bias{l}")
                nc.sync.dma_start(bt[:], bias_in[l][:, :])
                bias_sb[l] = bt

            featA = featp.tile([128, m.NBLK * 128], TD, tag="featA")
            nc.sync.dma_start(featA[:], xT_in[:, :])
            featB = featp.tile([128, m.NBLK * 128], TD, tag="featB")
            p1T = featp.tile([128, m.NBLK * 128], TD, tag="p1T")

            lib_inst = nc.gpsimd.load_library(library_config.mlp)
            lib_pin = lib_inst.ins
            creg = nc.gpsimd.alloc_register("gcnt")

            # pre-zero every gather buffer: skipped (padding) gather slots
            # must never expose uninitialized SBUF (NaN * 0 = NaN in the mm)
            for b in range(m.NBUCK):
                maxts = max((m.ts_sc[si][b]
                             for si in range(len(m.sc_blocks))), default=0)
                if maxts == 0:
                    continue
                nb = ebuf_bufs_last if b == m.NBUCK - 1 else ebuf_bufs
                for _ in range(nb):
                    z = epools[b].tile([128, maxts, 128], TD, tag=f"e{b}")
                    nc.vector.memset(z[:, :, :], 0.0)

            tbl_p1 = [[[dramp.tile([m.CHKS[k], m.C], TD,
                                   name=f"tblp1_{l}_r{r}_k{k}",
                                   addr_space="Shared",
                                   tag=f"tblp1_{l}_r{r}_k{k}")
                        for k in range(m.NAG)]
                       for l in range(3)] for r in range(repeat)]
            ag_p1 = [[dramp.tile([m.QRS[k], m.C], TD, name=f"agp1_{l}_k{k}",
                                 tag=f"agp1_{l}_k{k}") for k in range(m.NAG)]
                     for l in range(3)]
            tbl_h = [[[dramp.tile([m.CHKS[k], m.C], TD,
                                  name=f"tblh_{l}_r{r}_k{k}",
                                  addr_space="Shared",
                                  tag=f"tblh_{l}_r{r}_k{k}")
                       for k in range(m.NAG)]
                      for l in range(2)] for r in range(repeat)]
            ag_h = [[dramp.tile([m.QRS[k], m.C], TD, name=f"agh_{l}_k{k}",
                                tag=f"agh_{l}_k{k}") for k in range(m.NAG)]
                    for l in range(2)]

            def bucket_rows(tbl):
                out = []
                for b in range(m.NBUCK):
                    if isinstance(tbl, list):
                        out.append(tbl[b][0:m.CHKS[b], :])
                    else:
                        out.append(
                            tbl[m.CHKSTART[b]:m.CHKSTART[b + 1], :])
                return out

            def emit_rows(stg, blk, ag_tiles):
                r0 = blk * 128
                k = next(i for i in range(m.NAG)
                         if m.QSTART[i] <= r0 < m.QSTART[i + 1])
                lr0 = r0 - m.QSTART[k]
                nc.sync.dma_start(ag_tiles[k][lr0: lr0 + 128, :], stg[:])

            def emit_table_block(feat_sb, blk, ag_tiles):
                tp = tpp.tile([128, 128], TD, tag="tp")
                nc.tensor.transpose(
                    tp[:], feat_sb[:, blk * 128:(blk + 1) * 128], ident[:])
                stg = stagep.tile([128, 128], TD, tag="tstage")
                nc.scalar.activation(stg[:], tp[:], AF.Identity,
                                     scale=dinv_sb[:, blk: blk + 1])
                emit_rows(stg, blk, ag_tiles)

            def allgather_chunk(ag_tiles, tbl_tiles, k):
                if "ag" in skip:
                    return
                nc.gpsimd.collective_compute(
                    "AllGather", mybir.AluOpType.bypass,
                    replica_groups=groups,
                    ins=[ag_tiles[k][:, :].opt()],
                    outs=[tbl_tiles[k][:, :].opt()])

            # sc index after which chunk k's last table block has been
            # emitted (one extra sc of pipeline slack before triggering AG)
            _ag_after = {}
            for k in range(m.NAG):
                last_blk = m.QSTART[k + 1] // 128 - 1
                si_k = next(i for i, bl in enumerate(m.sc_blocks)
                            if last_blk in bl)
                _ag_after.setdefault(min(si_k + 1, len(m.sc_blocks) - 1),
                                     []).append(k)

            def dense_block(l, feat_in, p2_psum, blk):
                co = m.COUT if l == 2 else m.C
                cols = slice(blk * 128, (blk + 1) * 128)
                # p2_psum holds U2^T [node, f]; P2 = -dinv∘U2
                p2nf = stagep.tile([128, 128], TD, tag="p2nf")
                nc.scalar.activation(p2nf[:], p2_psum[:], AF.Identity,
                                     scale=dinvn_sb[:, blk: blk + 1])
                tpd = tpp.tile([128, 128], TD, tag="tp")
                nc.tensor.transpose(tpd[:], p2nf[:], ident[:])
                p2s = stagep.tile([128, 128], TD, tag="p2stage")
                nc.scalar.activation(p2s[:], tpd[:], AF.Identity)
                dn = dnp.tile([128, 128], F32, tag="dn")
                nc.tensor.matmul(dn[:co, :], wd_sb[(l, "A")][:, :],
                                 feat_in[:, cols], start=True, stop=False)
                nc.tensor.matmul(dn[:co, :], wd_sb[(l, "B")][:, :],
                                 p1T[:, cols], start=False, stop=False)
                nc.tensor.matmul(dn[:co, :], wd_sb[(l, "C")][:, :],
                                 p2s[:], start=False, stop=True)
                if l < 2:
                    outf = featB if l == 0 else featA
                    nc.scalar.activation(outf[:, cols], dn[:, :], AF.Relu,
                                         bias=bias_sb[l][:, :])
                    emit_table_block(outf, blk, ag_h[l])
                else:
                    stg = stagep.tile([m.COUT, 128], F32, tag="ostage")
                    nc.scalar.activation(stg[:], dn[:co, :], AF.Identity,
                                         bias=bias_sb[l][:co, :])
                    nc.sync.dma_start(out_dram[:, cols], stg[:])

            def prop(tbl_aps, out_feat=None, make_tbl_ag=None, dense=None,
                     ag_spec=None):
                nsc = len(m.sc_blocks)
                PRE = min(3, nsc - 1)
                idx_tiles = {}
                ebufs_si = {}

                def load_idx(si):
                    t = idxp.tile([128, max(8, max(m.idx_sc_cols))], I16,
                                  tag="idx")
                    if m.idx_sc_cols[si]:
                        nc.sync.dma_start(
                            t[:, : m.idx_sc_cols[si]],
                            idx_in[:, m.idx_sc_col[si]:
                                   m.idx_sc_col[si] + m.idx_sc_cols[si]])
                    idx_tiles[si] = t

                def emit_gathers(si, buckets):
                    idx_sb = idx_tiles[si]
                    ebufs = ebufs_si.setdefault(si, {})
                    for b in buckets:
                        ts = m.ts_sc[si][b]
                        if ts == 0:
                            continue
                        ebuf = epools[b].tile([128, ts, 128], TD, tag=f"e{b}")
                        c0 = m.idx_seg_col[si][b] - m.idx_sc_col[si]
                        cc0 = m.ccol[si][b] - m.idx_sc_col[si]
                        if "gather" in skip:
                            nc.vector.memset(ebuf[:, 0:1, :], 0.0)
                        for j, (cells, tk, boff) in (
                                enumerate(m.calls[si][b])
                                if "gather" not in skip else ()):
                            nc.gpsimd.reg_load(
                                creg, idx_sb[0:1, cc0 + j: cc0 + j + 1])
                            g = nc.gpsimd.dma_gather(
                                ebuf[:, boff: boff + tk, :], tbl_aps[b],
                                idx_sb[:, c0 + boff * 8: c0 + (boff + tk) * 8],
                                tk * 128, creg, m.C,
                                queue_num=0)
                            tile.add_dep_helper(lib_pin, g.ins, sync=False,
                                                reason="lib before gather")
                        ebufs[b] = ebuf

                # software-pipelined emission: buckets 0..B-2 of the next PRE
                # sc chunks are issued ahead, the last bucket (gated by the
                # last table AG chunk) just-in-time — so its sem wait can't
                # stall the in-order Pool queue ahead of independent gathers.
                early = list(range(m.NBUCK - 1))
                late = [m.NBUCK - 1]
                for si in range(PRE):
                    load_idx(si)
                    emit_gathers(si, early)
                for si, blks in enumerate(m.sc_blocks):
                    if PRE == 0:
                        load_idx(si)
                        emit_gathers(si, early + late)
                    else:
                        emit_gathers(si, late)
                        nxt = si + PRE
                        if nxt < nsc:
                            load_idx(nxt)
                            emit_gathers(nxt, early)
                    ebufs = ebufs_si[si]
                    for ci in range(0, len(blks), m.W2CHUNK):
                        cblks = blks[ci: ci + m.W2CHUNK]
                        ntile = sum(int(m.T[blk, b]) for blk in cblks
                                    for b in range(m.NBUCK))
                        if ntile == 0:
                            for blk in cblks:
                                _zero_block(nc, accp, out_feat, make_tbl_ag,
                                            dense, blk, emit_table_block,
                                            dense_block)
                            continue
                        g0 = min(m.mm_tile[(blk, b, 0)] for blk in cblks
                                 for b in range(m.NBUCK) if m.T[blk, b] > 0)
                        if w2_mode == "stream":
                            w2_sb = w2p.tile([128, ntile * 128], FP8,
                                             tag="w2")
                            nc.sync.dma_start(
                                w2_sb[:], w2_in[:, g0 * 128:(g0 + ntile) * 128])
                        else:
                            w2_sb = w2p.tile([128, ntile * 2], F32, tag="w2")
                            nc.sync.dma_start(
                                w2_sb[:], w2_in[:, g0 * 2:(g0 + ntile) * 2])
                        for blk in cblks:
                            n_mm = sum(int(m.T[blk, b])
                                       for b in range(m.NBUCK))
                            acc = accp.tile([128, 128], F32, tag="acc")
                            i = 0
                            if "mm" in skip:
                                nc.vector.memset(acc[:], 0.0)
                                n_mm = 0
                            for b in (range(m.NBUCK) if "mm" not in skip else ()):
                                for t in range(int(m.T[blk, b])):
                                    gt = m.mm_tile[(blk, b, t)] - g0
                                    et = m.eoff[si][b][blk] + t
                                    if w2_mode == "stream":
                                        rhs = w2_sb[:, gt * 128:(gt + 1) * 128]
                                    else:
                                        w2t = w2tp.tile([128, 128], TD,
                                                        tag="w2t")
                                        nc.vector.tensor_scalar(
                                            w2t[:], iota_sb[:],
                                            w2_sb[:, 2 * gt: 2 * gt + 1],
                                            w2_sb[:, 2 * gt + 1: 2 * gt + 2],
                                            op0=mybir.AluOpType.is_equal,
                                            op1=mybir.AluOpType.mult)
                                        rhs = w2t[:]
                                    nc.tensor.matmul(
                                        acc[:],
                                        rhs,
                                        ebufs[b][:, et, :],
                                        start=(i == 0), stop=(i == n_mm - 1))
                                    i += 1
                            if n_mm == 0:
                                nc.vector.memset(acc[:], 0.0)
                            if out_feat is not None:
                                # acc = U1^T [node, f]: p1T gets -dinv∘U1
                                # (transposed back), the table row gets
                                # -dinv^2∘U1 directly (already node-major)
                                p1nf = stagep.tile([128, 128], TD,
                                                   tag="p1nf")
                                nc.scalar.activation(
                                    p1nf[:], acc[:], AF.Identity,
                                    scale=dinvn_sb[:, blk: blk + 1])
                                tpq = tpp.tile([128, 128], TD, tag="tp")
                                nc.tensor.transpose(tpq[:], p1nf[:],
                                                    ident[:])
                                nc.scalar.activation(
                                    out_feat[:, blk * 128:(blk + 1) * 128],
                                    tpq[:], AF.Identity)
                            if make_tbl_ag is not None:
                                stg_t = stagep.tile([128, 128], TD,
                                                    tag="tstage")
                                nc.scalar.activation(
                                    stg_t[:], acc[:], AF.Identity,
                                    scale=dinvsq_sb[:, blk: blk + 1])
                                emit_rows(stg_t, blk, make_tbl_ag)
                            if dense is not None:
                                dense_block(dense[0], dense[1], acc, blk)
                    if ag_spec is not None:
                        for k in _ag_after.get(si, ()):
                            allgather_chunk(ag_spec[0], ag_spec[1], k)

            for rep in range(repeat):
                if rep > 0:
                    nc.sync.dma_start(featA[:], xT_in[:, :])
                for l in range(3):
                    feat_in = featA if l != 1 else featB
                    tbl_in = x_table if l == 0 else tbl_h[rep][l - 1]
                    prop(bucket_rows(tbl_in), out_feat=p1T,
                         make_tbl_ag=ag_p1[l],
                         ag_spec=(ag_p1[l], tbl_p1[rep][l]))
                    prop(bucket_rows(tbl_p1[rep][l]), dense=(l, feat_in),
                         ag_spec=((ag_h[l], tbl_h[rep][l])
                                  if l < 2 else None))

    # The runtime locks each DMASW completion sem to one SWDGE queue, and
    # the tile scheduler assigns DMASW lanes round-robin in *scheduled*
    # order — so pick each gather's queue from its assigned lane.
    for fblk in nc.m.functions[0].blocks:
        for i in fblk.instructions:
            if isinstance(i, mybir.InstDMAGatherAnt):
                sinfo = i.sync_info
                for u in (sinfo.on_update if sinfo else []):
                    nm = getattr(u, "ant_name", "") or ""
                    if nm.startswith("DMASW"):
                        i.queue_num = int(nm[5:].split("_")[0]) % nqueues

    nc.compile()
    return nc


def _zero_block(nc, accp, out_feat, make_tbl_ag, dense, blk,
                emit_table_block, dense_block):
    acc = accp.tile([128, 128], F32, tag="acc")
    nc.vector.memset(acc[:], 0.0)
    if out_feat is not None:
        nc.vector.tensor_copy(out_feat[:, blk * 128:(blk + 1) * 128], acc[:])
    if make_tbl_ag is not None:
        emit_table_block(out_feat, blk, make_tbl_ag)
    if dense is not None:
        dense_block(dense[0], dense[1], acc, blk)


def assemble_output(meta, results):
    m = meta
    out = np.zeros((m.N, m.COUT), dtype=np.float32)
    for c in range(m.NCORES):
        o = results[c]["outT"]
        out[c * m.NPC:(c + 1) * m.NPC, :] = o[:, m.perms[c]].T
    return out


def numpy_reference(x, edge_index, Ws, bs):
    src = np.asarray(edge_index[0], dtype=np.int64)
    dst = np.asarray(edge_index[1], dtype=np.int64)
    n = x.shape[0]
    deg = np.bincount(src, minlength=n).astype(np.float64)
    dinv = np.where(deg > 0, 1.0 / np.sqrt(np.maximum(deg, 1e-30)), 0.0)
    w = (-(dinv[src] * dinv[dst])).astype(np.float64)

    def prop(h):
        out = np.zeros_like(h)
        np.add.at(out, dst, w[:, None] * h[src])
        return out

    def cheb(h, W, b):
        Tx0, Tx1 = h, prop(h)
        out = Tx0 @ W[0] + Tx1 @ W[1]
        Tx2 = 2.0 * prop(Tx1) - Tx0
        out = out + Tx2 @ W[2]
        return out + b

    h = np.asarray(x, dtype=np.float64)
    h = np.maximum(cheb(h, Ws[0], bs[0]), 0.0)
    h = np.maximum(cheb(h, Ws[1], bs[1]), 0.0)
    return cheb(h, Ws[2], bs[2]).astype(np.float32)


# ---------------------------------------------------------------------------
# self-contained kernel entry point (full inputs in, full output out)
# ---------------------------------------------------------------------------

LAST_EXEC_NS = None
LAST_RESULTS = None


def kernel(**inputs):
    global LAST_EXEC_NS, LAST_RESULTS
    import numpy as _np
    from concourse.bass_utils import run_bass_kernel_spmd

    x = _np.asarray(inputs["x"], _np.float32)
    edge_index = _np.asarray(inputs["edge_index"], _np.int64)
    Ws = [_np.asarray(inputs[f"W{l}"], _np.float32) for l in range(3)]
    bs = [_np.asarray(inputs[f"b{l}"], _np.float32) for l in range(3)]

    meta = make_meta(100000, 128, 64, 8, edge_index)
    per_core = prep_inputs(meta, x, edge_index, Ws, bs)
    nc = build_nc(meta)
    import os
    trace = os.environ.get("GNN_TRACE", "0") == "1"
    try:
        res = run_bass_kernel_spmd(nc, per_core, list(range(meta.NCORES)),
                                   trace=trace)
    except Exception:
        if not trace:
            raise
        res = run_bass_kernel_spmd(nc, per_core, list(range(meta.NCORES)),
                                   trace=False)
    LAST_EXEC_NS = res.exec_time_ns
    LAST_RESULTS = res
    return assemble_output(meta, res.results)



# revision 33
# speedup vs baseline: 1.0007x; 1.0007x over previous
"""ChebNet (K=3, 3 layers) on trn2, node-sharded across 8 cores.

Math (per layer): out = h@(W0-W2) + P1@W1 + P2@(2*W2) + b,  P1 = L h, P2 = L P1,
L = -D^-1/2 A D^-1/2 (deg = out-degree over src).  relu after layers 0,1.

Device scheme per core (owns a contiguous slice of dst nodes):
 - features live transposed in SBUF: [128 feat (partitions), nodes (free)]
 - propagation: dma_gather rows of the (replicated, node-major) feature table
   for each edge (sorted by (block of 128 dst, bucket of <=32768 src rows)),
   128 edges -> 128 partitions; segmented sum via matmul:
     psum[f, d] += sum_e E_tile[e, f] * W2_tile[e, d]
   where W2_tile[e, d] = edge_weight if (dst_local % 128)==d else 0
   (host-precomputed, streamed from DRAM).
 - gather tables for the next prop are produced by PE-transposing each
   128-node block and AllGather-ing the node-major slices.
"""

from dataclasses import dataclass, field

import numpy as np
import ml_dtypes

import concourse.bass as bass
import concourse.bacc as bacc
import concourse.mybir as mybir
import concourse.tile as tile
from concourse import library_config
from concourse.tile import TileContext

BF16 = mybir.dt.bfloat16
F32 = mybir.dt.float32
I16 = mybir.dt.int16
AF = mybir.ActivationFunctionType
GMAXT = 4   # 128-idx tiles per dma_gather call (smaller calls pipeline better)


@dataclass
class Meta:
    N: int
    C: int          # in/hidden feature dim (must be 128)
    COUT: int
    NCORES: int
    SCB: int
    W2CHUNK: int    # blocks per w2 dma chunk
    NAG: int = 4    # allgather chunks (== src buckets)
    QRS: tuple = ()     # per-chunk per-core table rows (block-aligned)
    CHKS: tuple = ()    # per-chunk global table rows (= 8*QRS)
    QSTART: tuple = ()  # newlocal start of each quarter (block-aligned)
    OQSTART: tuple = () # orig-local start of each quarter
    CHKSTART: tuple = ()
    NPC: int = 0
    NPCP: int = 0       # padded per-core rows (NBLK*128, with holes)
    NBLK: int = 0
    NBUCK: int = 0
    perms: list = None  # [core] orig local -> newlocal (with holes)
    T: object = None              # [NBLK, NBUCK] tiles per (blk, bucket)
    sc_blocks: list = field(default_factory=list)
    TOT_TILES: int = 0
    TOT_IDXCOLS: int = 0
    ts_sc: list = None            # [sc][b] tiles in gather call
    idx_seg_col: list = None      # [sc][b] column offset of call segment
    idx_sc_col: list = None       # [sc] col base of sc idx chunk
    idx_sc_cols: list = None      # [sc] col count of sc idx chunk
    eoff: list = None             # [sc][b][blk] tile offset in (sc,b) buffer
    mm_tile: dict = None          # (blk,b,t) -> global tile index (MM order)
    GCALL: int = 8                # tiles per gather call (cell-aligned)
    calls: list = None            # [sc][b] list of (blk, t0, tk, boff)
    ccol: list = None             # [sc][b] idx col of per-call count entries

    def finalize(self):
        assert self.C == 128
        self.sc_blocks = [
            list(range(s, min(s + self.SCB, self.NBLK)))
            for s in range(0, self.NBLK, self.SCB)
        ]
        nsc = len(self.sc_blocks)
        self.ts_sc = [[0] * self.NBUCK for _ in range(nsc)]
        self.eoff = [[dict() for _ in range(self.NBUCK)] for _ in range(nsc)]
        self.mm_tile = {}
        g = 0
        for si, blks in enumerate(self.sc_blocks):
            for b in range(self.NBUCK):
                off = 0
                for blk in blks:
                    self.eoff[si][b][blk] = off
                    off += int(self.T[blk, b])
                self.ts_sc[si][b] = off
            for blk in blks:
                for b in range(self.NBUCK):
                    for t in range(int(self.T[blk, b])):
                        self.mm_tile[(blk, b, t)] = g
                        g += 1
        self.TOT_TILES = g
        # pack consecutive cells into calls of <= GCALL tiles; per-core valid
        # counts (via the count reg) trim each call to its last valid row,
        # interior-cell padding gathers row 0 harmlessly.
        # call = (cells, tk, boff); cell = (blk, t0, tcnt) tile-slice of a
        # (blk, b) group at ebuf offset boff.., tk total tiles.
        self.calls = [[[] for _ in range(self.NBUCK)] for _ in range(nsc)]
        for si, blks in enumerate(self.sc_blocks):
            for b in range(self.NBUCK):
                pieces = []
                for blk in blks:
                    Tc = int(self.T[blk, b])
                    for t0 in range(0, Tc, self.GCALL):
                        pieces.append((blk, t0, min(self.GCALL, Tc - t0)))
                group, gtiles = [], 0
                for piece in pieces:
                    if gtiles + piece[2] > self.GCALL and group:
                        self.calls[si][b].append(
                            (group, gtiles,
                             self.eoff[si][b][group[0][0]] + group[0][1]))
                        group, gtiles = [], 0
                    group.append(piece)
                    gtiles += piece[2]
                if group:
                    self.calls[si][b].append(
                        (group, gtiles,
                         self.eoff[si][b][group[0][0]] + group[0][1]))
        self.idx_seg_col = [[0] * self.NBUCK for _ in range(nsc)]
        self.ccol = [[0] * self.NBUCK for _ in range(nsc)]
        self.idx_sc_col = [0] * nsc
        self.idx_sc_cols = [0] * nsc
        col = 0
        for si in range(nsc):
            self.idx_sc_col[si] = col
            for b in range(self.NBUCK):
                self.idx_seg_col[si][b] = col
                col += self.ts_sc[si][b] * 8
                self.ccol[si][b] = col
                col += len(self.calls[si][b])
            self.idx_sc_cols[si] = col - self.idx_sc_col[si]
        self.TOT_IDXCOLS = col


def table_pos(m, v):
    """Node id -> row in the AG-chunk-major node table (post-balance)."""
    v = np.asarray(v, dtype=np.int64)
    c = v // m.NPC
    lr = v - c * m.NPC
    allperm = np.stack([m.perms[i] for i in range(m.NCORES)])
    nl = allperm[c, lr]
    q = np.searchsorted(np.asarray(m.QSTART), nl, side="right") - 1
    qs = np.asarray(m.QSTART)[q]
    return (np.asarray(m.CHKSTART)[q] + c * np.asarray(m.QRS)[q]
            + (nl - qs))


def _balance_quarter(indeg_q, nblocks, cap_reg=512, cap_ovf=1024):
    """Assign a quarter's nodes to its blocks s.t. per-(block, bucket) edge
    counts stay <= cap_reg (last block: cap_ovf). Returns block per node."""
    n_q = indeg_q.shape[0]
    order = np.argsort(-indeg_q.sum(1), kind="stable")
    cnt = np.zeros((nblocks, 4), np.int64)
    nn = np.zeros(nblocks, np.int64)
    caps = np.full((nblocks, 4), cap_reg, np.int64)
    caps[-1] = cap_ovf
    assign = np.empty(n_q, np.int64)
    for i in order:
        d = indeg_q[i]
        tot = cnt + d
        feas = (nn < 128) & (tot <= caps).all(1)
        score = (tot / caps).max(1)
        if feas.any():
            score = np.where(feas, score, np.inf)
        else:
            score = np.where(nn < 128, score, np.inf)
        blkk = int(np.argmin(score))
        assign[i] = blkk
        cnt[blkk] += d
        nn[blkk] += 1
    return assign


def make_meta(N, C, COUT, ncores, edge_index, scb=4, w2chunk=4,
              nag=4, gcall=8, balance=True, nblk_pad=6):
    m = Meta(N=N, C=C, COUT=COUT, NCORES=ncores, SCB=scb,
             W2CHUNK=w2chunk, NAG=nag, GCALL=gcall)
    assert nag == 4
    m.NPC = N // ncores
    assert m.NPC * ncores == N
    # pad the block count: hole slack lets the balancer hit <=512
    # edges per (block, bucket) cell (T=4, two cells per gather call)
    m.NBLK = (m.NPC + 127) // 128 + (nblk_pad if balance else 0)
    m.NBLK += (-m.NBLK) % nag
    m.NPCP = m.NBLK * 128
    m.NBUCK = nag
    bq = m.NBLK // nag
    nb = [bq] * nag
    m.QRS = tuple(x * 128 for x in nb)
    m.CHKS = tuple(ncores * x for x in m.QRS)
    m.QSTART = tuple(int(x) for x in np.concatenate(
        [[0], np.cumsum(m.QRS)]))[:5]
    m.CHKSTART = tuple(int(x) for x in np.concatenate(
        [[0], np.cumsum(m.CHKS)]))[:5]
    # orig-local quarter boundaries: equal shares of the real rows
    m.OQSTART = tuple(min(i * ((m.NPC + nag - 1) // nag), m.NPC)
                      for i in range(5))
    assert max(m.CHKS) <= 32768

    src = np.asarray(edge_index[0], dtype=np.int64)
    dst = np.asarray(edge_index[1], dtype=np.int64)
    oq_bounds = np.asarray(m.OQSTART[1:4])
    src_lr = src - (src // m.NPC) * m.NPC
    qsrc = np.searchsorted(oq_bounds, src_lr, side="right")  # 0..3

    # per (dst node, src quarter) in-degree
    indeg = np.bincount(dst * 4 + qsrc, minlength=N * 4).reshape(N, 4)

    m.perms = []
    for c in range(ncores):
        perm = np.empty(m.NPC, dtype=np.int64)
        for q in range(4):
            lo, hi = m.OQSTART[q], m.OQSTART[q + 1]
            nodes = np.arange(c * m.NPC + lo, c * m.NPC + hi)
            if balance:
                assign = _balance_quarter(indeg[nodes], nb[q])
            else:
                assign = (np.arange(hi - lo)) // 128
            # slot nodes within their assigned blocks
            slot = np.zeros(hi - lo, dtype=np.int64)
            for blkk in range(nb[q]):
                sel = np.nonzero(assign == blkk)[0]
                assert len(sel) <= 128, (c, q, blkk, len(sel))
                slot[sel] = np.arange(len(sel))
            perm[lo:hi] = m.QSTART[q] + assign * 128 + slot
        m.perms.append(perm)

    core = dst // m.NPC
    nl_d = np.stack(m.perms)[core, dst - core * m.NPC]
    blk = nl_d // 128
    lin = (core * m.NBLK + blk) * m.NBUCK + qsrc
    cnt = np.bincount(lin, minlength=ncores * m.NBLK * m.NBUCK).reshape(
        ncores, m.NBLK, m.NBUCK)
    m.T = np.ceil(cnt / 128.0).astype(np.int64).max(axis=0)
    m.finalize()
    return m


def prep_inputs(meta, x, edge_index, Ws, bs, table_dtype=ml_dtypes.bfloat16, w2_mode="stream"):
    """Returns per-core input dict list."""
    m = meta
    N, C = m.N, m.C
    src = np.asarray(edge_index[0], dtype=np.int64)
    dst = np.asarray(edge_index[1], dtype=np.int64)
    deg = np.bincount(src, minlength=N).astype(np.float64)
    dinv = np.where(deg > 0, 1.0 / np.sqrt(np.maximum(deg, 1e-30)), 0.0)
    w = (-(dinv[src] * dinv[dst])).astype(np.float32)

    x = np.asarray(x, dtype=np.float32)
    shared = {}
    tpos = table_pos(m, np.arange(N, dtype=np.int64))
    xt_tab = np.zeros((m.CHKSTART[4], C), dtype=table_dtype)
    # table rows are pre-scaled by dinv[src]; the -dinv[dst] side is applied
    # per-partition when prop outputs leave PSUM. The streamed "one-hot" is
    # then an exact 0/1 indicator (fp8-representable).
    xt_tab[tpos] = (x * dinv[:, None].astype(np.float32)).astype(table_dtype)
    shared["x_table"] = np.ascontiguousarray(xt_tab)
    shared["ident"] = np.eye(128, dtype=table_dtype)
    shared["iotat"] = np.ascontiguousarray(
        np.tile(np.arange(128, dtype=np.float32), (128, 1)).astype(table_dtype))
    for l in range(3):
        W = np.asarray(Ws[l], dtype=np.float32)
        shared[f"wA{l}"] = np.ascontiguousarray((W[0] - W[2]).astype(table_dtype))
        shared[f"wB{l}"] = np.ascontiguousarray(W[1].astype(table_dtype))
        shared[f"wC{l}"] = np.ascontiguousarray((2.0 * W[2]).astype(table_dtype))
        bias = np.zeros((128, 1), dtype=np.float32)
        bias[: bs[l].shape[0], 0] = np.asarray(bs[l], dtype=np.float32)
        shared[f"bias{l}"] = bias

    core = dst // m.NPC
    srcpos = table_pos(m, src)
    chk_bounds = np.asarray(m.CHKSTART[1:4])
    chk_start = np.asarray(m.CHKSTART[:4])
    per_core = []
    for c in range(m.NCORES):
        sel = np.nonzero(core == c)[0]
        s_c = srcpos[sel]
        nl_c = m.perms[c][dst[sel] - c * m.NPC]
        w_c = w[sel]
        blk_c = nl_c // 128
        buck_c = np.searchsorted(chk_bounds, s_c, side="right")
        dcol_c = nl_c % 128
        lidx_c = s_c - chk_start[buck_c]

        order = np.lexsort((buck_c, blk_c))
        blk_s, buck_s = blk_c[order], buck_c[order]
        lidx_s, w_s, dcol_s = lidx_c[order], w_c[order], dcol_c[order]
        grp = blk_s * m.NBUCK + buck_s
        starts = np.searchsorted(grp, np.arange(m.NBLK * m.NBUCK), side="left")
        ends = np.searchsorted(grp, np.arange(m.NBLK * m.NBUCK), side="right")
        ent = {}
        for blk in range(m.NBLK):
            for b in range(m.NBUCK):
                t = int(m.T[blk, b])
                if t == 0:
                    continue
                gid = blk * m.NBUCK + b
                s0, s1 = int(starts[gid]), int(ends[gid])
                L = t * 128
                n = s1 - s0
                assert n <= L, (n, L, blk, b)
                ei = np.full(L, -1, dtype=np.int16)
                ew = np.zeros(L, dtype=np.float32)
                ed = np.zeros(L, dtype=np.int64)
                ei[:n] = lidx_s[s0:s1].astype(np.int16)
                ew[:n] = w_s[s0:s1]
                ed[:n] = dcol_s[s0:s1]
                ent[(blk, b)] = (ei, ew, ed, n)

        # w2 stream [128, TOT_TILES*128] in MM order (stream mode) or
        # per-tile (off, w) scalars [128, 2*TOT_TILES] (dve mode)
        if w2_mode == "stream":
            w2 = np.zeros((128, m.TOT_TILES * 128), dtype=np.float32)
            p128 = np.arange(128)
            for (blk, b), (ei, ew, ed, n) in ent.items():
                for ti in range(int(m.T[blk, b])):
                    g = m.mm_tile[(blk, b, ti)]
                    valid = (np.arange(ti * 128, (ti + 1) * 128) < n)
                    w2[p128, g * 128 + ed[ti * 128:(ti + 1) * 128]] = \
                        valid.astype(np.float32)
        else:
            w2 = np.zeros((128, m.TOT_TILES * 2), dtype=np.float32)
            for (blk, b), (ei, ew, ed, n) in ent.items():
                for ti in range(int(m.T[blk, b])):
                    g = m.mm_tile[(blk, b, ti)]
                    w2[:, 2 * g] = ed[ti * 128:(ti + 1) * 128]
                    w2[:, 2 * g + 1] = ew[ti * 128:(ti + 1) * 128]
        # per-call valid counts: trim to the call's last valid row; padding
        # before that point gathers row 0 (idx 0), after it stays -1 so the
        # ucode's trailing-negative trim skips it per-core.
        call_cnt = {}
        for si in range(len(m.sc_blocks)):
            for b in range(m.NBUCK):
                for j, (cells, tk, boff) in enumerate(m.calls[si][b]):
                    cnt_j = 0
                    for (blk, t0, tcnt) in cells:
                        if (blk, b) not in ent:
                            continue
                        n = ent[(blk, b)][3]
                        v = min(max(n - t0 * 128, 0), tcnt * 128)
                        if v > 0:
                            off_rel = (m.eoff[si][b][blk] + t0 - boff) * 128
                            cnt_j = off_rel + v
                    if cnt_j == 0:
                        blk0, t00, _ = cells[0]
                        if (blk0, b) in ent:
                            ent[(blk0, b)][0][t00 * 128] = 0
                        cnt_j = 1
                    else:
                        for (blk, t0, tcnt) in cells:
                            if (blk, b) not in ent:
                                continue
                            ei, _, _, n = ent[(blk, b)]
                            off_rel = (m.eoff[si][b][blk] + t0 - boff) * 128
                            v = min(max(n - t0 * 128, 0), tcnt * 128)
                            lo = v
                            hi = min(tcnt * 128, cnt_j - off_rel)
                            if hi > lo:
                                ei[t0 * 128 + lo: t0 * 128 + hi] = 0
                    call_cnt[(si, b, j)] = cnt_j
        # idx stream [128, TOT_IDXCOLS] in gather-call order
        idxs = np.zeros((128, max(1, m.TOT_IDXCOLS)), dtype=np.int16)
        for si, blks in enumerate(m.sc_blocks):
            for b in range(m.NBUCK):
                seg = [ent[(blk, b)][0] for blk in blks if (blk, b) in ent]
                if seg:
                    seg = np.concatenate(seg)
                    cols = seg.reshape(-1, 16).T  # [16, L/16]
                    c0 = m.idx_seg_col[si][b]
                    idxs[:, c0: c0 + cols.shape[1]] = np.tile(cols, (8, 1))
                for j in range(len(m.calls[si][b])):
                    idxs[:, m.ccol[si][b] + j] = np.int16(
                        call_cnt[(si, b, j)])

        xT = np.zeros((128, m.NBLK * 128), dtype=table_dtype)
        xT[:C, m.perms[c]] = x[c * m.NPC:(c + 1) * m.NPC, :].T.astype(
            table_dtype)

        dv = np.zeros(m.NBLK * 128, dtype=np.float32)
        dv[m.perms[c]] = dinv[c * m.NPC:(c + 1) * m.NPC].astype(np.float32)
        dv = dv.reshape(m.NBLK, 128).T  # [dcol, blk]
        dinv_blk = np.ascontiguousarray(dv)
        dinvn_blk = np.ascontiguousarray(-dv)
        dinvsq_blk = np.ascontiguousarray(-dv * dv)

        d = dict(shared)
        fp8 = mybir.dt.np(mybir.dt.float8e4)
        d["w2"] = np.ascontiguousarray(
            w2.astype(fp8 if w2_mode == "stream" else np.float32))
        d["idxs"] = idxs
        d["xT"] = xT
        d["dinv_blk"] = dinv_blk
        d["dinvn_blk"] = dinvn_blk
        d["dinvsq_blk"] = dinvsq_blk
        per_core.append(d)
    return per_core


def build_nc(meta, table_mybir_dt=BF16, repeat=1, skip=(), ncores_override=None, gmaxt=None, nqueues=4, ebuf_bufs=5, ebuf_bufs_last=3, idx_bufs=5, w2_mode="stream", scratch=16384):
    m = meta
    ncores = ncores_override or m.NCORES
    gmaxt = gmaxt or GMAXT
    TD = table_mybir_dt
    nc = bacc.Bacc("TRN2", target_bir_lowering=False, debug=False,
                   num_devices=ncores, num_swdge_queues=4,
                   dynamic_dma_scratch_size=scratch)

    x_table = nc.dram_tensor("x_table", [m.CHKSTART[4], m.C], TD,
                             kind="ExternalInput")
    xT_in = nc.dram_tensor("xT", [128, m.NBLK * 128], TD, kind="ExternalInput")
    idx_in = nc.dram_tensor("idxs", [128, max(1, m.TOT_IDXCOLS)], I16,
                            kind="ExternalInput")
    assert w2_mode == "stream"
    FP8 = mybir.dt.float8e4
    w2_in = nc.dram_tensor("w2", [128, m.TOT_TILES * 128], FP8,
                           kind="ExternalInput")
    dinv_in = nc.dram_tensor("dinv_blk", [128, m.NBLK], F32,
                             kind="ExternalInput")
    dinvn_in = nc.dram_tensor("dinvn_blk", [128, m.NBLK], F32,
                              kind="ExternalInput")
    dinvsq_in = nc.dram_tensor("dinvsq_blk", [128, m.NBLK], F32,
                               kind="ExternalInput")
    iota_in = nc.dram_tensor("iotat", [128, 128], TD, kind="ExternalInput")
    ident_in = nc.dram_tensor("ident", [128, 128], TD, kind="ExternalInput")
    wd_in, bias_in = {}, {}
    for l in range(3):
        co = m.COUT if l == 2 else m.C
        for nm in ("A", "B", "C"):
            wd_in[(l, nm)] = nc.dram_tensor(f"w{nm}{l}", [128, co], TD,
                                            kind="ExternalInput")
        bias_in[l] = nc.dram_tensor(f"bias{l}", [128, 1], F32,
                                    kind="ExternalInput")
    out_dram = nc.dram_tensor("outT", [m.COUT, m.NBLK * 128], F32,
                              kind="ExternalOutput")

    groups = [list(range(ncores))]

    with TileContext(nc) as tc:
        with (
            tc.tile_pool(name="const", bufs=1) as constp,
            tc.tile_pool(name="feat", bufs=1) as featp,
            tc.tile_pool(name="idxp", bufs=idx_bufs) as idxp,
            tc.tile_pool(name="w2p", bufs=(3 if w2_mode != "stream" else 2)) as w2p,
            tc.tile_pool(name="e0", bufs=ebuf_bufs) as ep0,
            tc.tile_pool(name="e1", bufs=ebuf_bufs) as ep1,
            tc.tile_pool(name="e2", bufs=ebuf_bufs) as ep2,
            tc.tile_pool(name="e3", bufs=ebuf_bufs_last) as ep3,
            tc.tile_pool(name="stage", bufs=4) as stagep,
            tc.tile_pool(name="w2t", bufs=(6 if w2_mode != "stream" else 1)) as w2tp,
            tc.tile_pool(name="acc", bufs=4, space="PSUM") as accp,
            tc.tile_pool(name="tp", bufs=2, space="PSUM") as tpp,
            tc.tile_pool(name="dn", bufs=2, space="PSUM") as dnp,
            tc.tile_pool(name="dram", bufs=1, space="DRAM") as dramp,
        ):
            epools = [ep0, ep1, ep2, ep3]

            ident = constp.tile([128, 128], TD)
            nc.sync.dma_start(ident[:], ident_in[:, :])
            iota_sb = constp.tile([128, 128], TD)
            nc.sync.dma_start(iota_sb[:], iota_in[:, :])
            dinv_sb = constp.tile([128, m.NBLK], F32, tag="dinv")
            nc.sync.dma_start(dinv_sb[:], dinv_in[:, :])
            dinvn_sb = constp.tile([128, m.NBLK], F32, tag="dinvn")
            nc.sync.dma_start(dinvn_sb[:], dinvn_in[:, :])
            dinvsq_sb = constp.tile([128, m.NBLK], F32, tag="dinvsq")
            nc.sync.dma_start(dinvsq_sb[:], dinvsq_in[:, :])
            wd_sb, bias_sb = {}, {}
            for l in range(3):
                co = m.COUT if l == 2 else m.C
                for nm in ("A", "B", "C"):
                    t = constp.tile([128, co], TD, tag=f"w{nm}{l}")
                    nc.sync.dma_start(t[:], wd_in[(l, nm)][:, :])
                    wd_sb[(l, nm)] = t
                bt = constp.tile([128, 1], F32, tag=f"bias{l}")
                nc.sync.dma_start(bt[:], bias_in[l][:, :])
                bias_sb[l] = bt

            featA = featp.tile([128, m.NBLK * 128], TD, tag="featA")
            nc.sync.dma_start(featA[:], xT_in[:, :])
            featB = featp.tile([128, m.NBLK * 128], TD, tag="featB")
            p1T = featp.tile([128, m.NBLK * 128], TD, tag="p1T")

            lib_inst = nc.gpsimd.load_library(library_config.mlp)
            lib_pin = lib_inst.ins
            creg = nc.gpsimd.alloc_register("gcnt")

            # pre-zero every gather buffer: skipped (padding) gather slots
            # must never expose uninitialized SBUF (NaN * 0 = NaN in the mm)
            for b in range(m.NBUCK):
                maxts = max((m.ts_sc[si][b]
                             for si in range(len(m.sc_blocks))), default=0)
                if maxts == 0:
                    continue
                nb = ebuf_bufs_last if b == m.NBUCK - 1 else ebuf_bufs
                for _ in range(nb):
                    z = epools[b].tile([128, maxts, 128], TD, tag=f"e{b}")
                    nc.vector.memset(z[:, :, :], 0.0)

            tbl_p1 = [[[dramp.tile([m.CHKS[k], m.C], TD,
                                   name=f"tblp1_{l}_r{r}_k{k}",
                                   addr_space="Shared",
                                   tag=f"tblp1_{l}_r{r}_k{k}")
                        for k in range(m.NAG)]
                       for l in range(3)] for r in range(repeat)]
            ag_p1 = [[dramp.tile([m.QRS[k], m.C], TD, name=f"agp1_{l}_k{k}",
                                 tag=f"agp1_{l}_k{k}") for k in range(m.NAG)]
                     for l in range(3)]
            tbl_h = [[[dramp.tile([m.CHKS[k], m.C], TD,
                                  name=f"tblh_{l}_r{r}_k{k}",
                                  addr_space="Shared",
                                  tag=f"tblh_{l}_r{r}_k{k}")
                       for k in range(m.NAG)]
                      for l in range(2)] for r in range(repeat)]
            ag_h = [[dramp.tile([m.QRS[k], m.C], TD, name=f"agh_{l}_k{k}",
                                tag=f"agh_{l}_k{k}") for k in range(m.NAG)]
                    for l in range(2)]

            def bucket_rows(tbl):
                out = []
                for b in range(m.NBUCK):
                    if isinstance(tbl, list):
                        out.append(tbl[b][0:m.CHKS[b], :])
                    else:
                        out.append(
                            tbl[m.CHKSTART[b]:m.CHKSTART[b + 1], :])
                return out

            def emit_rows(stg, blk, ag_tiles):
                r0 = blk * 128
                k = next(i for i in range(m.NAG)
                         if m.QSTART[i] <= r0 < m.QSTART[i + 1])
                lr0 = r0 - m.QSTART[k]
                nc.sync.dma_start(ag_tiles[k][lr0: lr0 + 128, :], stg[:])

            def emit_table_block(feat_sb, blk, ag_tiles):
                tp = tpp.tile([128, 128], TD, tag="tp")
                nc.tensor.transpose(
                    tp[:], feat_sb[:, blk * 128:(blk + 1) * 128], ident[:])
                stg = stagep.tile([128, 128], TD, tag="tstage")
                nc.scalar.activation(stg[:], tp[:], AF.Identity,
                                     scale=dinv_sb[:, blk: blk + 1])
                emit_rows(stg, blk, ag_tiles)

            def allgather_chunk(ag_tiles, tbl_tiles, k):
                if "ag" in skip:
                    return
                nc.gpsimd.collective_compute(
                    "AllGather", mybir.AluOpType.bypass,
                    replica_groups=groups,
                    ins=[ag_tiles[k][:, :].opt()],
                    outs=[tbl_tiles[k][:, :].opt()])

            # sc index after which chunk k's last table block has been
            # emitted (one extra sc of pipeline slack before triggering AG)
            _ag_after = {}
            for k in range(m.NAG):
                last_blk = m.QSTART[k + 1] // 128 - 1
                si_k = next(i for i, bl in enumerate(m.sc_blocks)
                            if last_blk in bl)
                _ag_after.setdefault(min(si_k + 1, len(m.sc_blocks) - 1),
                                     []).append(k)

            def dense_block(l, feat_in, p2_psum, blk):
                co = m.COUT if l == 2 else m.C
                cols = slice(blk * 128, (blk + 1) * 128)
                # p2_psum holds U2^T [node, f]; P2 = -dinv∘U2
                p2nf = stagep.tile([128, 128], TD, tag="p2nf")
                nc.scalar.activation(p2nf[:], p2_psum[:], AF.Identity,
                                     scale=dinvn_sb[:, blk: blk + 1])
                tpd = tpp.tile([128, 128], TD, tag="tp")
                nc.tensor.transpose(tpd[:], p2nf[:], ident[:])
                p2s = stagep.tile([128, 128], TD, tag="p2stage")
                nc.scalar.activation(p2s[:], tpd[:], AF.Identity)
                dn = dnp.tile([128, 128], F32, tag="dn")
                nc.tensor.matmul(dn[:co, :], wd_sb[(l, "A")][:, :],
                                 feat_in[:, cols], start=True, stop=False)
                nc.tensor.matmul(dn[:co, :], wd_sb[(l, "B")][:, :],
                                 p1T[:, cols], start=False, stop=False)
                nc.tensor.matmul(dn[:co, :], wd_sb[(l, "C")][:, :],
                                 p2s[:], start=False, stop=True)
                if l < 2:
                    outf = featB if l == 0 else featA
                    nc.scalar.activation(outf[:, cols], dn[:, :], AF.Relu,
                                         bias=bias_sb[l][:, :])
                    emit_table_block(outf, blk, ag_h[l])
                else:
                    stg = stagep.tile([m.COUT, 128], F32, tag="ostage")
                    nc.scalar.activation(stg[:], dn[:co, :], AF.Identity,
                                         bias=bias_sb[l][:co, :])
                    nc.sync.dma_start(out_dram[:, cols], stg[:])

            def prop(tbl_aps, out_feat=None, make_tbl_ag=None, dense=None,
                     ag_spec=None):
                nsc = len(m.sc_blocks)
                PRE = min(4, nsc - 1)
                idx_tiles = {}
                ebufs_si = {}

                def load_idx(si):
                    t = idxp.tile([128, max(8, max(m.idx_sc_cols))], I16,
                                  tag="idx")
                    if m.idx_sc_cols[si]:
                        nc.sync.dma_start(
                            t[:, : m.idx_sc_cols[si]],
                            idx_in[:, m.idx_sc_col[si]:
                                   m.idx_sc_col[si] + m.idx_sc_cols[si]])
                    idx_tiles[si] = t

                def emit_gathers(si, buckets):
                    idx_sb = idx_tiles[si]
                    ebufs = ebufs_si.setdefault(si, {})
                    for b in buckets:
                        ts = m.ts_sc[si][b]
                        if ts == 0:
                            continue
                        ebuf = epools[b].tile([128, ts, 128], TD, tag=f"e{b}")
                        c0 = m.idx_seg_col[si][b] - m.idx_sc_col[si]
                        cc0 = m.ccol[si][b] - m.idx_sc_col[si]
                        if "gather" in skip:
                            nc.vector.memset(ebuf[:, 0:1, :], 0.0)
                        for j, (cells, tk, boff) in (
                                enumerate(m.calls[si][b])
                                if "gather" not in skip else ()):
                            nc.gpsimd.reg_load(
                                creg, idx_sb[0:1, cc0 + j: cc0 + j + 1])
                            g = nc.gpsimd.dma_gather(
                                ebuf[:, boff: boff + tk, :], tbl_aps[b],
                                idx_sb[:, c0 + boff * 8: c0 + (boff + tk) * 8],
                                tk * 128, creg, m.C,
                                queue_num=0)
                            tile.add_dep_helper(lib_pin, g.ins, sync=False,
                                                reason="lib before gather")
                        ebufs[b] = ebuf

                # software-pipelined emission: buckets 0..B-2 of the next PRE
                # sc chunks are issued ahead, the last bucket (gated by the
                # last table AG chunk) just-in-time — so its sem wait can't
                # stall the in-order Pool queue ahead of independent gathers.
                early = list(range(m.NBUCK - 1))
                late = [m.NBUCK - 1]
                for si in range(PRE):
                    load_idx(si)
                    emit_gathers(si, early)
                for si, blks in enumerate(m.sc_blocks):
                    if PRE == 0:
                        load_idx(si)
                        emit_gathers(si, early + late)
                    else:
                        emit_gathers(si, late)
                        nxt = si + PRE
                        if nxt < nsc:
                            load_idx(nxt)
                            emit_gathers(nxt, early)
                    ebufs = ebufs_si[si]
                    for ci in range(0, len(blks), m.W2CHUNK):
                        cblks = blks[ci: ci + m.W2CHUNK]
                        ntile = sum(int(m.T[blk, b]) for blk in cblks
                                    for b in range(m.NBUCK))
                        if ntile == 0:
                            for blk in cblks:
                                _zero_block(nc, accp, out_feat, make_tbl_ag,
                                            dense, blk, emit_table_block,
                                            dense_block)
                            continue
                        g0 = min(m.mm_tile[(blk, b, 0)] for blk in cblks
                                 for b in range(m.NBUCK) if m.T[blk, b] > 0)
                        if w2_mode == "stream":
                            w2_sb = w2p.tile([128, ntile * 128], FP8,
                                             tag="w2")
                            nc.sync.dma_start(
                                w2_sb[:], w2_in[:, g0 * 128:(g0 + ntile) * 128])
                        else:
                            w2_sb = w2p.tile([128, ntile * 2], F32, tag="w2")
                            nc.sync.dma_start(
                                w2_sb[:], w2_in[:, g0 * 2:(g0 + ntile) * 2])
                        for blk in cblks:
                            n_mm = sum(int(m.T[blk, b])
                                       for b in range(m.NBUCK))
                            acc = accp.tile([128, 128], F32, tag="acc")
                            i = 0
                            if "mm" in skip:
                                nc.vector.memset(acc[:], 0.0)
                                n_mm = 0
                            for b in (range(m.NBUCK) if "mm" not in skip else ()):
                                for t in range(int(m.T[blk, b])):
                                    gt = m.mm_tile[(blk, b, t)] - g0
                                    et = m.eoff[si][b][blk] + t
                                    if w2_mode == "stream":
                                        rhs = w2_sb[:, gt * 128:(gt + 1) * 128]
                                    else:
                                        w2t = w2tp.tile([128, 128], TD,
                                                        tag="w2t")
                                        nc.vector.tensor_scalar(
                                            w2t[:], iota_sb[:],
                                            w2_sb[:, 2 * gt: 2 * gt + 1],
                                            w2_sb[:, 2 * gt + 1: 2 * gt + 2],
                                            op0=mybir.AluOpType.is_equal,
                                            op1=mybir.AluOpType.mult)
                                        rhs = w2t[:]
                                    nc.tensor.matmul(
                                        acc[:],
                                        rhs,
                                        ebufs[b][:, et, :],
                                        start=(i == 0), stop=(i == n_mm - 1))
                                    i += 1
                            if n_mm == 0:
                                nc.vector.memset(acc[:], 0.0)
                            if out_feat is not None:
                                # acc = U1^T [node, f]: p1T gets -dinv∘U1
                                # (transposed back), the table row gets
                                # -dinv^2∘U1 directly (already node-major)
                                p1nf = stagep.tile([128, 128], TD,
                                                   tag="p1nf")
                                nc.scalar.activation(
                                    p1nf[:], acc[:], AF.Identity,
                                    scale=dinvn_sb[:, blk: blk + 1])
                                tpq = tpp.tile([128, 128], TD, tag="tp")
                                nc.tensor.transpose(tpq[:], p1nf[:],
                                                    ident[:])
                                nc.scalar.activation(
                                    out_feat[:, blk * 128:(blk + 1) * 128],
                                    tpq[:], AF.Identity)
                            if make_tbl_ag is not None:
                                stg_t = stagep.tile([128, 128], TD,
                                                    tag="tstage")
                                nc.scalar.activation(
                                    stg_t[:], acc[:], AF.Identity,
                                    scale=dinvsq_sb[:, blk: blk + 1])
                                emit_rows(stg_t, blk, make_tbl_ag)
                            if dense is not None:
                                dense_block(dense[0], dense[1], acc, blk)
                    if ag_spec is not None:
                        for k in _ag_after.get(si, ()):
                            allgather_chunk(ag_spec[0], ag_spec[1], k)

            for rep in range(repeat):
                if rep > 0:
                    nc.sync.dma_start(featA[:], xT_in[:, :])
                for l in range(3):
                    feat_in = featA if l != 1 else featB
                    tbl_in = x_table if l == 0 else tbl_h[rep][l - 1]
                    prop(bucket_rows(tbl_in), out_feat=p1T,
                         make_tbl_ag=ag_p1[l],
                         ag_spec=(ag_p1[l], tbl_p1[rep][l]))
                    prop(bucket_rows(tbl_p1[rep][l]), dense=(l, feat_in),
                         ag_spec=((ag_h[l], tbl_h[rep][l])
                                  if l < 2 else None))

    # The runtime locks each DMASW completion sem to one SWDGE queue, and
    # the tile scheduler assigns DMASW lanes round-robin in *scheduled*
    # order — so pick each gather's queue from its assigned lane.
    for fblk in nc.m.functions[0].blocks:
        for i in fblk.instructions:
            if isinstance(i, mybir.InstDMAGatherAnt):
                sinfo = i.sync_info
                for u in (sinfo.on_update if sinfo else []):
                    nm = getattr(u, "ant_name", "") or ""
                    if nm.startswith("DMASW"):
                        i.queue_num = int(nm[5:].split("_")[0]) % nqueues

    nc.compile()
    return nc


def _zero_block(nc, accp, out_feat, make_tbl_ag, dense, blk,
                emit_table_block, dense_block):
    acc = accp.tile([128, 128], F32, tag="acc")
    nc.vector.memset(acc[:], 0.0)
    if out_feat is not None:
        nc.vector.tensor_copy(out_feat[:, blk * 128:(blk + 1) * 128], acc[:])
    if make_tbl_ag is not None:
        emit_table_block(out_feat, blk, make_tbl_ag)
    if dense is not None:
        dense_block(dense[0], dense[1], acc, blk)


def assemble_output(meta, results):
    m = meta
    out = np.zeros((m.N, m.COUT), dtype=np.float32)
    for c in range(m.NCORES):
        o = results[c]["outT"]
        out[c * m.NPC:(c + 1) * m.NPC, :] = o[:, m.perms[c]].T
    return out


def numpy_reference(x, edge_index, Ws, bs):
    src = np.asarray(edge_index[0], dtype=np.int64)
    dst = np.asarray(edge_index[1], dtype=np.int64)
    n = x.shape[0]
    deg = np.bincount(src, minlength=n).astype(np.float64)
    dinv = np.where(deg > 0, 1.0 / np.sqrt(np.maximum(deg, 1e-30)), 0.0)
    w = (-(dinv[src] * dinv[dst])).astype(np.float64)

    def prop(h):
        out = np.zeros_like(h)
        np.add.at(out, dst, w[:, None] * h[src])
        return out

    def cheb(h, W, b):
        Tx0, Tx1 = h, prop(h)
        out = Tx0 @ W[0] + Tx1 @ W[1]
        Tx2 = 2.0 * prop(Tx1) - Tx0
        out = out + Tx2 @ W[2]
        return out + b

    h = np.asarray(x, dtype=np.float64)
    h = np.maximum(cheb(h, Ws[0], bs[0]), 0.0)
    h = np.maximum(cheb(h, Ws[1], bs[1]), 0.0)
    return cheb(h, Ws[2], bs[2]).astype(np.float32)


# ---------------------------------------------------------------------------
# self-contained kernel entry point (full inputs in, full output out)
# ---------------------------------------------------------------------------

LAST_EXEC_NS = None
LAST_RESULTS = None


def kernel(**inputs):
    global LAST_EXEC_NS, LAST_RESULTS
    import numpy as _np
    from concourse.bass_utils import run_bass_kernel_spmd

    x = _np.asarray(inputs["x"], _np.float32)
    edge_index = _np.asarray(inputs["edge_index"], _np.int64)
    Ws = [_np.asarray(inputs[f"W{l}"], _np.float32) for l in range(3)]
    bs = [_np.asarray(inputs[f"b{l}"], _np.float32) for l in range(3)]

    meta = make_meta(100000, 128, 64, 8, edge_index)
    per_core = prep_inputs(meta, x, edge_index, Ws, bs)
    nc = build_nc(meta)
    import os
    trace = os.environ.get("GNN_TRACE", "0") == "1"
    try:
        res = run_bass_kernel_spmd(nc, per_core, list(range(meta.NCORES)),
                                   trace=trace)
    except Exception:
        if not trace:
            raise
        res = run_bass_kernel_spmd(nc, per_core, list(range(meta.NCORES)),
                                   trace=False)
    LAST_EXEC_NS = res.exec_time_ns
    LAST_RESULTS = res
    return assemble_output(meta, res.results)



# revision 34
# speedup vs baseline: 1.0297x; 1.0289x over previous
"""ChebNet (K=3, 3 layers) on trn2, node-sharded across 8 cores.

Math (per layer): out = h@(W0-W2) + P1@W1 + P2@(2*W2) + b,  P1 = L h, P2 = L P1,
L = -D^-1/2 A D^-1/2 (deg = out-degree over src).  relu after layers 0,1.

Device scheme per core (owns a contiguous slice of dst nodes):
 - features live transposed in SBUF: [128 feat (partitions), nodes (free)]
 - propagation: dma_gather rows of the (replicated, node-major) feature table
   for each edge (sorted by (block of 128 dst, bucket of <=32768 src rows)),
   128 edges -> 128 partitions; segmented sum via matmul:
     psum[f, d] += sum_e E_tile[e, f] * W2_tile[e, d]
   where W2_tile[e, d] = edge_weight if (dst_local % 128)==d else 0
   (host-precomputed, streamed from DRAM).
 - gather tables for the next prop are produced by PE-transposing each
   128-node block and AllGather-ing the node-major slices.
"""

from dataclasses import dataclass, field

import numpy as np
import ml_dtypes

import concourse.bass as bass
import concourse.bacc as bacc
import concourse.mybir as mybir
import concourse.tile as tile
from concourse import library_config
from concourse.tile import TileContext

BF16 = mybir.dt.bfloat16
F32 = mybir.dt.float32
I16 = mybir.dt.int16
AF = mybir.ActivationFunctionType
GMAXT = 4   # 128-idx tiles per dma_gather call (smaller calls pipeline better)


@dataclass
class Meta:
    N: int
    C: int          # in/hidden feature dim (must be 128)
    COUT: int
    NCORES: int
    SCB: int
    W2CHUNK: int    # blocks per w2 dma chunk
    NAG: int = 4    # allgather chunks (== src buckets)
    QRS: tuple = ()     # per-chunk per-core table rows (block-aligned)
    CHKS: tuple = ()    # per-chunk global table rows (= 8*QRS)
    QSTART: tuple = ()  # newlocal start of each quarter (block-aligned)
    OQSTART: tuple = () # orig-local start of each quarter
    CHKSTART: tuple = ()
    NPC: int = 0
    NPCP: int = 0       # padded per-core rows (NBLK*128, with holes)
    NBLK: int = 0
    NBUCK: int = 0
    perms: list = None  # [core] orig local -> newlocal (with holes)
    T: object = None              # [NBLK, NBUCK] tiles per (blk, bucket)
    sc_blocks: list = field(default_factory=list)
    TOT_TILES: int = 0
    TOT_IDXCOLS: int = 0
    ts_sc: list = None            # [sc][b] tiles in gather call
    idx_seg_col: list = None      # [sc][b] column offset of call segment
    idx_sc_col: list = None       # [sc] col base of sc idx chunk
    idx_sc_cols: list = None      # [sc] col count of sc idx chunk
    eoff: list = None             # [sc][b][blk] tile offset in (sc,b) buffer
    mm_tile: dict = None          # (blk,b,t) -> global tile index (MM order)
    GCALL: int = 8                # tiles per gather call (cell-aligned)
    calls: list = None            # [sc][b] list of (blk, t0, tk, boff)
    ccol: list = None             # [sc][b] idx col of per-call count entries

    def finalize(self):
        assert self.C == 128
        self.sc_blocks = [
            list(range(s, min(s + self.SCB, self.NBLK)))
            for s in range(0, self.NBLK, self.SCB)
        ]
        nsc = len(self.sc_blocks)
        self.ts_sc = [[0] * self.NBUCK for _ in range(nsc)]
        self.eoff = [[dict() for _ in range(self.NBUCK)] for _ in range(nsc)]
        self.mm_tile = {}
        g = 0
        for si, blks in enumerate(self.sc_blocks):
            for b in range(self.NBUCK):
                off = 0
                for blk in blks:
                    self.eoff[si][b][blk] = off
                    off += int(self.T[blk, b])
                self.ts_sc[si][b] = off
            for blk in blks:
                for b in range(self.NBUCK):
                    for t in range(int(self.T[blk, b])):
                        self.mm_tile[(blk, b, t)] = g
                        g += 1
        self.TOT_TILES = g
        # pack consecutive cells into calls of <= GCALL tiles; per-core valid
        # counts (via the count reg) trim each call to its last valid row,
        # interior-cell padding gathers row 0 harmlessly.
        # call = (cells, tk, boff); cell = (blk, t0, tcnt) tile-slice of a
        # (blk, b) group at ebuf offset boff.., tk total tiles.
        self.calls = [[[] for _ in range(self.NBUCK)] for _ in range(nsc)]
        for si, blks in enumerate(self.sc_blocks):
            for b in range(self.NBUCK):
                pieces = []
                for blk in blks:
                    Tc = int(self.T[blk, b])
                    for t0 in range(0, Tc, self.GCALL):
                        pieces.append((blk, t0, min(self.GCALL, Tc - t0)))
                group, gtiles = [], 0
                for piece in pieces:
                    if gtiles + piece[2] > self.GCALL and group:
                        self.calls[si][b].append(
                            (group, gtiles,
                             self.eoff[si][b][group[0][0]] + group[0][1]))
                        group, gtiles = [], 0
                    group.append(piece)
                    gtiles += piece[2]
                if group:
                    self.calls[si][b].append(
                        (group, gtiles,
                         self.eoff[si][b][group[0][0]] + group[0][1]))
        self.idx_seg_col = [[0] * self.NBUCK for _ in range(nsc)]
        self.ccol = [[0] * self.NBUCK for _ in range(nsc)]
        self.idx_sc_col = [0] * nsc
        self.idx_sc_cols = [0] * nsc
        col = 0
        for si in range(nsc):
            self.idx_sc_col[si] = col
            for b in range(self.NBUCK):
                self.idx_seg_col[si][b] = col
                col += self.ts_sc[si][b] * 8
                self.ccol[si][b] = col
                col += len(self.calls[si][b])
            self.idx_sc_cols[si] = col - self.idx_sc_col[si]
        self.TOT_IDXCOLS = col


def table_pos(m, v):
    """Node id -> row in the AG-chunk-major node table (post-balance)."""
    v = np.asarray(v, dtype=np.int64)
    c = v // m.NPC
    lr = v - c * m.NPC
    allperm = np.stack([m.perms[i] for i in range(m.NCORES)])
    nl = allperm[c, lr]
    q = np.searchsorted(np.asarray(m.QSTART), nl, side="right") - 1
    qs = np.asarray(m.QSTART)[q]
    return (np.asarray(m.CHKSTART)[q] + c * np.asarray(m.QRS)[q]
            + (nl - qs))


def _balance_quarter(indeg_q, nblocks, cap_reg=512, cap_ovf=1024):
    """Assign a quarter's nodes to its blocks s.t. per-(block, bucket) edge
    counts stay <= cap_reg (last block: cap_ovf). Returns block per node."""
    n_q = indeg_q.shape[0]
    order = np.argsort(-indeg_q.sum(1), kind="stable")
    cnt = np.zeros((nblocks, 4), np.int64)
    nn = np.zeros(nblocks, np.int64)
    caps = np.full((nblocks, 4), cap_reg, np.int64)
    caps[-1] = cap_ovf
    assign = np.empty(n_q, np.int64)
    for i in order:
        d = indeg_q[i]
        tot = cnt + d
        feas = (nn < 128) & (tot <= caps).all(1)
        score = (tot / caps).max(1)
        if feas.any():
            score = np.where(feas, score, np.inf)
        else:
            score = np.where(nn < 128, score, np.inf)
        blkk = int(np.argmin(score))
        assign[i] = blkk
        cnt[blkk] += d
        nn[blkk] += 1
    return assign


def make_meta(N, C, COUT, ncores, edge_index, scb=4, w2chunk=4,
              nag=4, gcall=8, balance=True, nblk_pad=6):
    m = Meta(N=N, C=C, COUT=COUT, NCORES=ncores, SCB=scb,
             W2CHUNK=w2chunk, NAG=nag, GCALL=gcall)
    assert nag == 4
    m.NPC = N // ncores
    assert m.NPC * ncores == N
    # pad the block count: hole slack lets the balancer hit <=512
    # edges per (block, bucket) cell (T=4, two cells per gather call)
    m.NBLK = (m.NPC + 127) // 128 + (nblk_pad if balance else 0)
    m.NBLK += (-m.NBLK) % nag
    m.NPCP = m.NBLK * 128
    m.NBUCK = nag
    bq = m.NBLK // nag
    nb = [bq] * nag
    m.QRS = tuple(x * 128 for x in nb)
    m.CHKS = tuple(ncores * x for x in m.QRS)
    m.QSTART = tuple(int(x) for x in np.concatenate(
        [[0], np.cumsum(m.QRS)]))[:5]
    m.CHKSTART = tuple(int(x) for x in np.concatenate(
        [[0], np.cumsum(m.CHKS)]))[:5]
    # orig-local quarter boundaries: equal shares of the real rows
    m.OQSTART = tuple(min(i * ((m.NPC + nag - 1) // nag), m.NPC)
                      for i in range(5))
    assert max(m.CHKS) <= 32768

    src = np.asarray(edge_index[0], dtype=np.int64)
    dst = np.asarray(edge_index[1], dtype=np.int64)
    oq_bounds = np.asarray(m.OQSTART[1:4])
    src_lr = src - (src // m.NPC) * m.NPC
    qsrc = np.searchsorted(oq_bounds, src_lr, side="right")  # 0..3

    # per (dst node, src quarter) in-degree
    indeg = np.bincount(dst * 4 + qsrc, minlength=N * 4).reshape(N, 4)

    m.perms = []
    for c in range(ncores):
        perm = np.empty(m.NPC, dtype=np.int64)
        for q in range(4):
            lo, hi = m.OQSTART[q], m.OQSTART[q + 1]
            nodes = np.arange(c * m.NPC + lo, c * m.NPC + hi)
            if balance:
                assign = _balance_quarter(indeg[nodes], nb[q])
            else:
                assign = (np.arange(hi - lo)) // 128
            # slot nodes within their assigned blocks
            slot = np.zeros(hi - lo, dtype=np.int64)
            for blkk in range(nb[q]):
                sel = np.nonzero(assign == blkk)[0]
                assert len(sel) <= 128, (c, q, blkk, len(sel))
                slot[sel] = np.arange(len(sel))
            perm[lo:hi] = m.QSTART[q] + assign * 128 + slot
        m.perms.append(perm)

    core = dst // m.NPC
    nl_d = np.stack(m.perms)[core, dst - core * m.NPC]
    blk = nl_d // 128
    lin = (core * m.NBLK + blk) * m.NBUCK + qsrc
    cnt = np.bincount(lin, minlength=ncores * m.NBLK * m.NBUCK).reshape(
        ncores, m.NBLK, m.NBUCK)
    m.T = np.ceil(cnt / 128.0).astype(np.int64).max(axis=0)
    m.finalize()
    return m


def prep_inputs(meta, x, edge_index, Ws, bs, table_dtype=ml_dtypes.bfloat16, w2_mode="stream"):
    """Returns per-core input dict list."""
    m = meta
    N, C = m.N, m.C
    src = np.asarray(edge_index[0], dtype=np.int64)
    dst = np.asarray(edge_index[1], dtype=np.int64)
    deg = np.bincount(src, minlength=N).astype(np.float64)
    dinv = np.where(deg > 0, 1.0 / np.sqrt(np.maximum(deg, 1e-30)), 0.0)
    w = (-(dinv[src] * dinv[dst])).astype(np.float32)

    x = np.asarray(x, dtype=np.float32)
    shared = {}
    tpos = table_pos(m, np.arange(N, dtype=np.int64))
    xt_tab = np.zeros((m.CHKSTART[4], C), dtype=table_dtype)
    # table rows are pre-scaled by dinv[src]; the -dinv[dst] side is applied
    # per-partition when prop outputs leave PSUM. The streamed "one-hot" is
    # then an exact 0/1 indicator (fp8-representable).
    xt_tab[tpos] = (x * dinv[:, None].astype(np.float32)).astype(table_dtype)
    shared["x_table"] = np.ascontiguousarray(xt_tab)
    shared["ident"] = np.eye(128, dtype=table_dtype)
    shared["iotat"] = np.ascontiguousarray(
        np.tile(np.arange(128, dtype=np.float32), (128, 1)).astype(table_dtype))
    for l in range(3):
        W = np.asarray(Ws[l], dtype=np.float32)
        shared[f"wA{l}"] = np.ascontiguousarray((W[0] - W[2]).astype(table_dtype))
        shared[f"wB{l}"] = np.ascontiguousarray(W[1].astype(table_dtype))
        shared[f"wC{l}"] = np.ascontiguousarray((2.0 * W[2]).astype(table_dtype))
        bias = np.zeros((128, 1), dtype=np.float32)
        bias[: bs[l].shape[0], 0] = np.asarray(bs[l], dtype=np.float32)
        shared[f"bias{l}"] = bias

    core = dst // m.NPC
    srcpos = table_pos(m, src)
    chk_bounds = np.asarray(m.CHKSTART[1:4])
    chk_start = np.asarray(m.CHKSTART[:4])
    per_core = []
    for c in range(m.NCORES):
        sel = np.nonzero(core == c)[0]
        s_c = srcpos[sel]
        nl_c = m.perms[c][dst[sel] - c * m.NPC]
        w_c = w[sel]
        blk_c = nl_c // 128
        buck_c = np.searchsorted(chk_bounds, s_c, side="right")
        dcol_c = nl_c % 128
        lidx_c = s_c - chk_start[buck_c]

        order = np.lexsort((buck_c, blk_c))
        blk_s, buck_s = blk_c[order], buck_c[order]
        lidx_s, w_s, dcol_s = lidx_c[order], w_c[order], dcol_c[order]
        grp = blk_s * m.NBUCK + buck_s
        starts = np.searchsorted(grp, np.arange(m.NBLK * m.NBUCK), side="left")
        ends = np.searchsorted(grp, np.arange(m.NBLK * m.NBUCK), side="right")
        ent = {}
        for blk in range(m.NBLK):
            for b in range(m.NBUCK):
                t = int(m.T[blk, b])
                if t == 0:
                    continue
                gid = blk * m.NBUCK + b
                s0, s1 = int(starts[gid]), int(ends[gid])
                L = t * 128
                n = s1 - s0
                assert n <= L, (n, L, blk, b)
                ei = np.full(L, -1, dtype=np.int16)
                ew = np.zeros(L, dtype=np.float32)
                ed = np.zeros(L, dtype=np.int64)
                ei[:n] = lidx_s[s0:s1].astype(np.int16)
                ew[:n] = w_s[s0:s1]
                ed[:n] = dcol_s[s0:s1]
                ent[(blk, b)] = (ei, ew, ed, n)

        # w2 stream [128, TOT_TILES*128] in MM order (stream mode) or
        # per-tile (off, w) scalars [128, 2*TOT_TILES] (dve mode)
        if w2_mode == "stream":
            w2 = np.zeros((128, m.TOT_TILES * 128), dtype=np.float32)
            p128 = np.arange(128)
            for (blk, b), (ei, ew, ed, n) in ent.items():
                for ti in range(int(m.T[blk, b])):
                    g = m.mm_tile[(blk, b, ti)]
                    valid = (np.arange(ti * 128, (ti + 1) * 128) < n)
                    w2[p128, g * 128 + ed[ti * 128:(ti + 1) * 128]] = \
                        valid.astype(np.float32)
        else:
            w2 = np.zeros((128, m.TOT_TILES * 2), dtype=np.float32)
            for (blk, b), (ei, ew, ed, n) in ent.items():
                for ti in range(int(m.T[blk, b])):
                    g = m.mm_tile[(blk, b, ti)]
                    w2[:, 2 * g] = ed[ti * 128:(ti + 1) * 128]
                    w2[:, 2 * g + 1] = ew[ti * 128:(ti + 1) * 128]
        # per-call valid counts: trim to the call's last valid row; padding
        # before that point gathers row 0 (idx 0), after it stays -1 so the
        # ucode's trailing-negative trim skips it per-core.
        call_cnt = {}
        for si in range(len(m.sc_blocks)):
            for b in range(m.NBUCK):
                for j, (cells, tk, boff) in enumerate(m.calls[si][b]):
                    cnt_j = 0
                    for (blk, t0, tcnt) in cells:
                        if (blk, b) not in ent:
                            continue
                        n = ent[(blk, b)][3]
                        v = min(max(n - t0 * 128, 0), tcnt * 128)
                        if v > 0:
                            off_rel = (m.eoff[si][b][blk] + t0 - boff) * 128
                            cnt_j = off_rel + v
                    if cnt_j == 0:
                        blk0, t00, _ = cells[0]
                        if (blk0, b) in ent:
                            ent[(blk0, b)][0][t00 * 128] = 0
                        cnt_j = 1
                    else:
                        for (blk, t0, tcnt) in cells:
                            if (blk, b) not in ent:
                                continue
                            ei, _, _, n = ent[(blk, b)]
                            off_rel = (m.eoff[si][b][blk] + t0 - boff) * 128
                            v = min(max(n - t0 * 128, 0), tcnt * 128)
                            lo = v
                            hi = min(tcnt * 128, cnt_j - off_rel)
                            if hi > lo:
                                ei[t0 * 128 + lo: t0 * 128 + hi] = 0
                    call_cnt[(si, b, j)] = cnt_j
        # idx stream [128, TOT_IDXCOLS] in gather-call order
        idxs = np.zeros((128, max(1, m.TOT_IDXCOLS)), dtype=np.int16)
        for si, blks in enumerate(m.sc_blocks):
            for b in range(m.NBUCK):
                seg = [ent[(blk, b)][0] for blk in blks if (blk, b) in ent]
                if seg:
                    seg = np.concatenate(seg)
                    cols = seg.reshape(-1, 16).T  # [16, L/16]
                    c0 = m.idx_seg_col[si][b]
                    idxs[:, c0: c0 + cols.shape[1]] = np.tile(cols, (8, 1))
                for j in range(len(m.calls[si][b])):
                    idxs[:, m.ccol[si][b] + j] = np.int16(
                        call_cnt[(si, b, j)])

        xT = np.zeros((128, m.NBLK * 128), dtype=table_dtype)
        xT[:C, m.perms[c]] = x[c * m.NPC:(c + 1) * m.NPC, :].T.astype(
            table_dtype)

        dv = np.zeros(m.NBLK * 128, dtype=np.float32)
        dv[m.perms[c]] = dinv[c * m.NPC:(c + 1) * m.NPC].astype(np.float32)
        dv = dv.reshape(m.NBLK, 128).T  # [dcol, blk]
        dinv_blk = np.ascontiguousarray(dv)
        dinvn_blk = np.ascontiguousarray(-dv)
        dinvsq_blk = np.ascontiguousarray(-dv * dv)

        d = dict(shared)
        fp8 = mybir.dt.np(mybir.dt.float8e4)
        d["w2"] = np.ascontiguousarray(
            w2.astype(fp8 if w2_mode == "stream" else np.float32))
        d["idxs"] = idxs
        d["xT"] = xT
        d["dinv_blk"] = dinv_blk
        d["dinvn_blk"] = dinvn_blk
        d["dinvsq_blk"] = dinvsq_blk
        per_core.append(d)
    return per_core


def build_nc(meta, table_mybir_dt=BF16, repeat=1, skip=(), ncores_override=None, gmaxt=None, nqueues=4, ebuf_bufs=5, ebuf_bufs_last=3, idx_bufs=5, w2_mode="stream", scratch=16384):
    m = meta
    ncores = ncores_override or m.NCORES
    gmaxt = gmaxt or GMAXT
    TD = table_mybir_dt
    nc = bacc.Bacc("TRN2", target_bir_lowering=False, debug=False,
                   num_devices=ncores, num_swdge_queues=4,
                   dynamic_dma_scratch_size=scratch)

    x_table = nc.dram_tensor("x_table", [m.CHKSTART[4], m.C], TD,
                             kind="ExternalInput")
    xT_in = nc.dram_tensor("xT", [128, m.NBLK * 128], TD, kind="ExternalInput")
    idx_in = nc.dram_tensor("idxs", [128, max(1, m.TOT_IDXCOLS)], I16,
                            kind="ExternalInput")
    assert w2_mode == "stream"
    FP8 = mybir.dt.float8e4
    w2_in = nc.dram_tensor("w2", [128, m.TOT_TILES * 128], FP8,
                           kind="ExternalInput")
    dinv_in = nc.dram_tensor("dinv_blk", [128, m.NBLK], F32,
                             kind="ExternalInput")
    dinvn_in = nc.dram_tensor("dinvn_blk", [128, m.NBLK], F32,
                              kind="ExternalInput")
    dinvsq_in = nc.dram_tensor("dinvsq_blk", [128, m.NBLK], F32,
                               kind="ExternalInput")
    iota_in = nc.dram_tensor("iotat", [128, 128], TD, kind="ExternalInput")
    ident_in = nc.dram_tensor("ident", [128, 128], TD, kind="ExternalInput")
    wd_in, bias_in = {}, {}
    for l in range(3):
        co = m.COUT if l == 2 else m.C
        for nm in ("A", "B", "C"):
            wd_in[(l, nm)] = nc.dram_tensor(f"w{nm}{l}", [128, co], TD,
                                            kind="ExternalInput")
        bias_in[l] = nc.dram_tensor(f"bias{l}", [128, 1], F32,
                                    kind="ExternalInput")
    out_dram = nc.dram_tensor("outT", [m.COUT, m.NBLK * 128], F32,
                              kind="ExternalOutput")

    groups = [list(range(ncores))]

    with TileContext(nc) as tc:
        with (
            tc.tile_pool(name="const", bufs=1) as constp,
            tc.tile_pool(name="feat", bufs=1) as featp,
            tc.tile_pool(name="idxp", bufs=idx_bufs) as idxp,
            tc.tile_pool(name="w2p", bufs=(3 if w2_mode != "stream" else 2)) as w2p,
            tc.tile_pool(name="e0", bufs=ebuf_bufs) as ep0,
            tc.tile_pool(name="e1", bufs=ebuf_bufs) as ep1,
            tc.tile_pool(name="e2", bufs=ebuf_bufs) as ep2,
            tc.tile_pool(name="e3", bufs=ebuf_bufs_last) as ep3,
            tc.tile_pool(name="stage", bufs=4) as stagep,
            tc.tile_pool(name="w2t", bufs=(6 if w2_mode != "stream" else 1)) as w2tp,
            tc.tile_pool(name="acc", bufs=4, space="PSUM") as accp,
            tc.tile_pool(name="tp", bufs=2, space="PSUM") as tpp,
            tc.tile_pool(name="dn", bufs=2, space="PSUM") as dnp,
            tc.tile_pool(name="dram", bufs=1, space="DRAM") as dramp,
        ):
            epools = [ep0, ep1, ep2, ep3]

            ident = constp.tile([128, 128], TD)
            nc.sync.dma_start(ident[:], ident_in[:, :])
            iota_sb = constp.tile([128, 128], TD)
            nc.sync.dma_start(iota_sb[:], iota_in[:, :])
            dinv_sb = constp.tile([128, m.NBLK], F32, tag="dinv")
            nc.sync.dma_start(dinv_sb[:], dinv_in[:, :])
            dinvn_sb = constp.tile([128, m.NBLK], F32, tag="dinvn")
            nc.sync.dma_start(dinvn_sb[:], dinvn_in[:, :])
            dinvsq_sb = constp.tile([128, m.NBLK], F32, tag="dinvsq")
            nc.sync.dma_start(dinvsq_sb[:], dinvsq_in[:, :])
            wd_sb, bias_sb = {}, {}
            for l in range(3):
                co = m.COUT if l == 2 else m.C
                for nm in ("A", "B", "C"):
                    t = constp.tile([128, co], TD, tag=f"w{nm}{l}")
                    nc.sync.dma_start(t[:], wd_in[(l, nm)][:, :])
                    wd_sb[(l, nm)] = t
                bt = constp.tile([128, 1], F32, tag=f"bias{l}")
                nc.sync.dma_start(bt[:], bias_in[l][:, :])
                bias_sb[l] = bt

            featA = featp.tile([128, m.NBLK * 128], TD, tag="featA")
            nc.sync.dma_start(featA[:], xT_in[:, :])
            featB = featp.tile([128, m.NBLK * 128], TD, tag="featB")
            p1T = featp.tile([128, m.NBLK * 128], TD, tag="p1T")

            lib_inst = nc.gpsimd.load_library(library_config.mlp)
            lib_pin = lib_inst.ins
            creg = nc.gpsimd.alloc_register("gcnt")

            # pre-zero every gather buffer: skipped (padding) gather slots
            # must never expose uninitialized SBUF (NaN * 0 = NaN in the mm)
            for b in range(m.NBUCK):
                maxts = max((m.ts_sc[si][b]
                             for si in range(len(m.sc_blocks))), default=0)
                if maxts == 0:
                    continue
                nb = ebuf_bufs_last if b == m.NBUCK - 1 else ebuf_bufs
                for _ in range(nb):
                    z = epools[b].tile([128, maxts, 128], TD, tag=f"e{b}")
                    nc.vector.memset(z[:, :, :], 0.0)

            tbl_p1 = [[[dramp.tile([m.CHKS[k], m.C], TD,
                                   name=f"tblp1_{l}_r{r}_k{k}",
                                   addr_space="Shared",
                                   tag=f"tblp1_{l}_r{r}_k{k}")
                        for k in range(m.NAG)]
                       for l in range(3)] for r in range(repeat)]
            ag_p1 = [[dramp.tile([m.QRS[k], m.C], TD, name=f"agp1_{l}_k{k}",
                                 tag=f"agp1_{l}_k{k}") for k in range(m.NAG)]
                     for l in range(3)]
            tbl_h = [[[dramp.tile([m.CHKS[k], m.C], TD,
                                  name=f"tblh_{l}_r{r}_k{k}",
                                  addr_space="Shared",
                                  tag=f"tblh_{l}_r{r}_k{k}")
                       for k in range(m.NAG)]
                      for l in range(2)] for r in range(repeat)]
            ag_h = [[dramp.tile([m.QRS[k], m.C], TD, name=f"agh_{l}_k{k}",
                                tag=f"agh_{l}_k{k}") for k in range(m.NAG)]
                    for l in range(2)]

            def bucket_rows(tbl):
                out = []
                for b in range(m.NBUCK):
                    if isinstance(tbl, list):
                        out.append(tbl[b][0:m.CHKS[b], :])
                    else:
                        out.append(
                            tbl[m.CHKSTART[b]:m.CHKSTART[b + 1], :])
                return out

            def emit_rows(stg, blk, ag_tiles):
                r0 = blk * 128
                k = next(i for i in range(m.NAG)
                         if m.QSTART[i] <= r0 < m.QSTART[i + 1])
                lr0 = r0 - m.QSTART[k]
                nc.sync.dma_start(ag_tiles[k][lr0: lr0 + 128, :], stg[:])

            def emit_table_block(feat_sb, blk, ag_tiles):
                tp = tpp.tile([128, 128], TD, tag="tp")
                nc.tensor.transpose(
                    tp[:], feat_sb[:, blk * 128:(blk + 1) * 128], ident[:])
                stg = stagep.tile([128, 128], TD, tag="tstage")
                nc.scalar.activation(stg[:], tp[:], AF.Identity,
                                     scale=dinv_sb[:, blk: blk + 1])
                emit_rows(stg, blk, ag_tiles)

            def allgather_chunk(ag_tiles, tbl_tiles, k):
                if "ag" in skip:
                    return
                nc.gpsimd.collective_compute(
                    "AllGather", mybir.AluOpType.bypass,
                    replica_groups=groups,
                    ins=[ag_tiles[k][:, :].opt()],
                    outs=[tbl_tiles[k][:, :].opt()])

            # sc index after which chunk k's last table block has been
            # emitted (one extra sc of pipeline slack before triggering AG)
            _ag_after = {}
            for k in range(m.NAG):
                last_blk = m.QSTART[k + 1] // 128 - 1
                si_k = next(i for i, bl in enumerate(m.sc_blocks)
                            if last_blk in bl)
                _ag_after.setdefault(min(si_k + 1, len(m.sc_blocks) - 1),
                                     []).append(k)

            def dense_block(l, feat_in, p2_psum, blk):
                co = m.COUT if l == 2 else m.C
                cols = slice(blk * 128, (blk + 1) * 128)
                # p2_psum holds U2^T [node, f]; P2 = -dinv∘U2
                p2nf = stagep.tile([128, 128], TD, tag="p2nf")
                nc.scalar.activation(p2nf[:], p2_psum[:], AF.Identity,
                                     scale=dinvn_sb[:, blk: blk + 1])
                tpd = tpp.tile([128, 128], TD, tag="tp")
                nc.tensor.transpose(tpd[:], p2nf[:], ident[:])
                p2s = stagep.tile([128, 128], TD, tag="p2stage")
                nc.scalar.activation(p2s[:], tpd[:], AF.Identity)
                dn = dnp.tile([128, 128], F32, tag="dn")
                nc.tensor.matmul(dn[:co, :], wd_sb[(l, "A")][:, :],
                                 feat_in[:, cols], start=True, stop=False)
                nc.tensor.matmul(dn[:co, :], wd_sb[(l, "B")][:, :],
                                 p1T[:, cols], start=False, stop=False)
                nc.tensor.matmul(dn[:co, :], wd_sb[(l, "C")][:, :],
                                 p2s[:], start=False, stop=True)
                if l < 2:
                    outf = featB if l == 0 else featA
                    nc.scalar.activation(outf[:, cols], dn[:, :], AF.Relu,
                                         bias=bias_sb[l][:, :])
                    emit_table_block(outf, blk, ag_h[l])
                else:
                    stg = stagep.tile([m.COUT, 128], F32, tag="ostage")
                    nc.scalar.activation(stg[:], dn[:co, :], AF.Identity,
                                         bias=bias_sb[l][:co, :])
                    nc.sync.dma_start(out_dram[:, cols], stg[:])

            def prop(tbl_aps, out_feat=None, make_tbl_ag=None, dense=None,
                     ag_spec=None):
                nsc = len(m.sc_blocks)
                PRE = min(3, nsc - 1)
                idx_tiles = {}
                ebufs_si = {}

                def load_idx(si):
                    t = idxp.tile([128, max(8, max(m.idx_sc_cols))], I16,
                                  tag="idx")
                    if m.idx_sc_cols[si]:
                        nc.sync.dma_start(
                            t[:, : m.idx_sc_cols[si]],
                            idx_in[:, m.idx_sc_col[si]:
                                   m.idx_sc_col[si] + m.idx_sc_cols[si]])
                    idx_tiles[si] = t

                def emit_gathers(si, buckets):
                    idx_sb = idx_tiles[si]
                    ebufs = ebufs_si.setdefault(si, {})
                    for b in buckets:
                        ts = m.ts_sc[si][b]
                        if ts == 0:
                            continue
                        ebuf = epools[b].tile([128, ts, 128], TD, tag=f"e{b}")
                        c0 = m.idx_seg_col[si][b] - m.idx_sc_col[si]
                        cc0 = m.ccol[si][b] - m.idx_sc_col[si]
                        if "gather" in skip:
                            nc.vector.memset(ebuf[:, 0:1, :], 0.0)
                        for j, (cells, tk, boff) in (
                                enumerate(m.calls[si][b])
                                if "gather" not in skip else ()):
                            nc.gpsimd.reg_load(
                                creg, idx_sb[0:1, cc0 + j: cc0 + j + 1])
                            g = nc.gpsimd.dma_gather(
                                ebuf[:, boff: boff + tk, :], tbl_aps[b],
                                idx_sb[:, c0 + boff * 8: c0 + (boff + tk) * 8],
                                tk * 128, creg, m.C,
                                queue_num=0)
                            tile.add_dep_helper(lib_pin, g.ins, sync=False,
                                                reason="lib before gather")
                        ebufs[b] = ebuf

                # software-pipelined emission: buckets 0..B-2 of the next PRE
                # sc chunks are issued ahead, the last bucket (gated by the
                # last table AG chunk) just-in-time — so its sem wait can't
                # stall the in-order Pool queue ahead of independent gathers.
                early = list(range(m.NBUCK - 1))
                late = [m.NBUCK - 1]
                for si in range(PRE):
                    load_idx(si)
                    emit_gathers(si, early)
                for si, blks in enumerate(m.sc_blocks):
                    if PRE == 0:
                        load_idx(si)
                        emit_gathers(si, early + late)
                    else:
                        emit_gathers(si, late)
                        nxt = si + PRE
                        if nxt < nsc:
                            load_idx(nxt)
                            emit_gathers(nxt, early)
                    ebufs = ebufs_si[si]
                    for ci in range(0, len(blks), m.W2CHUNK):
                        cblks = blks[ci: ci + m.W2CHUNK]
                        ntile = sum(int(m.T[blk, b]) for blk in cblks
                                    for b in range(m.NBUCK))
                        if ntile == 0:
                            for blk in cblks:
                                _zero_block(nc, accp, out_feat, make_tbl_ag,
                                            dense, blk, emit_table_block,
                                            dense_block)
                            continue
                        g0 = min(m.mm_tile[(blk, b, 0)] for blk in cblks
                                 for b in range(m.NBUCK) if m.T[blk, b] > 0)
                        if w2_mode == "stream":
                            w2_sb = w2p.tile([128, ntile * 128], FP8,
                                             tag="w2")
                            nc.sync.dma_start(
                                w2_sb[:], w2_in[:, g0 * 128:(g0 + ntile) * 128])
                        else:
                            w2_sb = w2p.tile([128, ntile * 2], F32, tag="w2")
                            nc.sync.dma_start(
                                w2_sb[:], w2_in[:, g0 * 2:(g0 + ntile) * 2])
                        for blk in cblks:
                            n_mm = sum(int(m.T[blk, b])
                                       for b in range(m.NBUCK))
                            acc = accp.tile([128, 128], F32, tag="acc")
                            i = 0
                            if "mm" in skip:
                                nc.vector.memset(acc[:], 0.0)
                                n_mm = 0
                            for b in (range(m.NBUCK) if "mm" not in skip else ()):
                                for t in range(int(m.T[blk, b])):
                                    gt = m.mm_tile[(blk, b, t)] - g0
                                    et = m.eoff[si][b][blk] + t
                                    if w2_mode == "stream":
                                        rhs = w2_sb[:, gt * 128:(gt + 1) * 128]
                                    else:
                                        w2t = w2tp.tile([128, 128], TD,
                                                        tag="w2t")
                                        nc.vector.tensor_scalar(
                                            w2t[:], iota_sb[:],
                                            w2_sb[:, 2 * gt: 2 * gt + 1],
                                            w2_sb[:, 2 * gt + 1: 2 * gt + 2],
                                            op0=mybir.AluOpType.is_equal,
                                            op1=mybir.AluOpType.mult)
                                        rhs = w2t[:]
                                    nc.tensor.matmul(
                                        acc[:],
                                        rhs,
                                        ebufs[b][:, et, :],
                                        start=(i == 0), stop=(i == n_mm - 1))
                                    i += 1
                            if n_mm == 0:
                                nc.vector.memset(acc[:], 0.0)
                            if out_feat is not None:
                                # acc = U1^T [node, f]: p1T gets -dinv∘U1
                                # (transposed back), the table row gets
                                # -dinv^2∘U1 directly (already node-major)
                                p1nf = stagep.tile([128, 128], TD,
                                                   tag="p1nf")
                                nc.scalar.activation(
                                    p1nf[:], acc[:], AF.Identity,
                                    scale=dinvn_sb[:, blk: blk + 1])
                                tpq = tpp.tile([128, 128], TD, tag="tp")
                                nc.tensor.transpose(tpq[:], p1nf[:],
                                                    ident[:])
                                nc.scalar.activation(
                                    out_feat[:, blk * 128:(blk + 1) * 128],
                                    tpq[:], AF.Identity)
                            if make_tbl_ag is not None:
                                stg_t = stagep.tile([128, 128], TD,
                                                    tag="tstage")
                                nc.scalar.activation(
                                    stg_t[:], acc[:], AF.Identity,
                                    scale=dinvsq_sb[:, blk: blk + 1])
                                emit_rows(stg_t, blk, make_tbl_ag)
                            if dense is not None:
                                dense_block(dense[0], dense[1], acc, blk)
                    if ag_spec is not None:
                        for k in _ag_after.get(si, ()):
                            allgather_chunk(ag_spec[0], ag_spec[1], k)

            for rep in range(repeat):
                if rep > 0:
                    nc.sync.dma_start(featA[:], xT_in[:, :])
                for l in range(3):
                    feat_in = featA if l != 1 else featB
                    tbl_in = x_table if l == 0 else tbl_h[rep][l - 1]
                    prop(bucket_rows(tbl_in), out_feat=p1T,
                         make_tbl_ag=ag_p1[l],
                         ag_spec=(ag_p1[l], tbl_p1[rep][l]))
                    prop(bucket_rows(tbl_p1[rep][l]), dense=(l, feat_in),
                         ag_spec=((ag_h[l], tbl_h[rep][l])
                                  if l < 2 else None))

    # The runtime locks each DMASW completion sem to one SWDGE queue, and
    # the tile scheduler assigns DMASW lanes round-robin in *scheduled*
    # order — so pick each gather's queue from its assigned lane.
    for fblk in nc.m.functions[0].blocks:
        for i in fblk.instructions:
            if isinstance(i, mybir.InstDMAGatherAnt):
                sinfo = i.sync_info
                for u in (sinfo.on_update if sinfo else []):
                    nm = getattr(u, "ant_name", "") or ""
                    if nm.startswith("DMASW"):
                        i.queue_num = int(nm[5:].split("_")[0]) % nqueues

    nc.compile()
    return nc


def _zero_block(nc, accp, out_feat, make_tbl_ag, dense, blk,
                emit_table_block, dense_block):
    acc = accp.tile([128, 128], F32, tag="acc")
    nc.vector.memset(acc[:], 0.0)
    if out_feat is not None:
        nc.vector.tensor_copy(out_feat[:, blk * 128:(blk + 1) * 128], acc[:])
    if make_tbl_ag is not None:
        emit_table_block(out_feat, blk, make_tbl_ag)
    if dense is not None:
        dense_block(dense[0], dense[1], acc, blk)


def assemble_output(meta, results):
    m = meta
    out = np.zeros((m.N, m.COUT), dtype=np.float32)
    for c in range(m.NCORES):
        o = results[c]["outT"]
        out[c * m.NPC:(c + 1) * m.NPC, :] = o[:, m.perms[c]].T
    return out


def numpy_reference(x, edge_index, Ws, bs):
    src = np.asarray(edge_index[0], dtype=np.int64)
    dst = np.asarray(edge_index[1], dtype=np.int64)
    n = x.shape[0]
    deg = np.bincount(src, minlength=n).astype(np.float64)
    dinv = np.where(deg > 0, 1.0 / np.sqrt(np.maximum(deg, 1e-30)), 0.0)
    w = (-(dinv[src] * dinv[dst])).astype(np.float64)

    def prop(h):
        out = np.zeros_like(h)
        np.add.at(out, dst, w[:, None] * h[src])
        return out

    def cheb(h, W, b):
        Tx0, Tx1 = h, prop(h)
        out = Tx0 @ W[0] + Tx1 @ W[1]
        Tx2 = 2.0 * prop(Tx1) - Tx0
        out = out + Tx2 @ W[2]
        return out + b

    h = np.asarray(x, dtype=np.float64)
    h = np.maximum(cheb(h, Ws[0], bs[0]), 0.0)
    h = np.maximum(cheb(h, Ws[1], bs[1]), 0.0)
    return cheb(h, Ws[2], bs[2]).astype(np.float32)


# ---------------------------------------------------------------------------
# self-contained kernel entry point (full inputs in, full output out)
# ---------------------------------------------------------------------------

LAST_EXEC_NS = None
LAST_RESULTS = None


def kernel(**inputs):
    global LAST_EXEC_NS, LAST_RESULTS
    import numpy as _np
    from concourse.bass_utils import run_bass_kernel_spmd

    x = _np.asarray(inputs["x"], _np.float32)
    edge_index = _np.asarray(inputs["edge_index"], _np.int64)
    Ws = [_np.asarray(inputs[f"W{l}"], _np.float32) for l in range(3)]
    bs = [_np.asarray(inputs[f"b{l}"], _np.float32) for l in range(3)]

    meta = make_meta(100000, 128, 64, 8, edge_index)
    per_core = prep_inputs(meta, x, edge_index, Ws, bs)
    nc = build_nc(meta)
    import os
    trace = os.environ.get("GNN_TRACE", "0") == "1"
    try:
        res = run_bass_kernel_spmd(nc, per_core, list(range(meta.NCORES)),
                                   trace=trace)
    except Exception:
        if not trace:
            raise
        res = run_bass_kernel_spmd(nc, per_core, list(range(meta.NCORES)),
                                   trace=False)
    LAST_EXEC_NS = res.exec_time_ns
    LAST_RESULTS = res
    return assemble_output(meta, res.results)

